# revision 17
# baseline (speedup 1.0000x reference)
"""3-layer GAT on 8 TRN2 NeuronCores.

Strategy (1D vertex-cut, dst-sharded):
  * Nodes are permuted: degree-sorted, dealt round-robin to 8 cores, so each
    core owns a contiguous range of NC=6272 "new" node ids whose windows of
    128 consecutive ids have near-uniform in-degree.
  * Per (core, window) the edge list is laid out as K[w] "slots" x 128 dst
    rows (shared K schedule across cores, padding slots have log_ew=-1e30).
  * Per layer, per window: indirect-DMA gather of h[src] rows (bf16) and
    als[src] (f32), attention weight w = exp(leaky(als+ald)+log_ew),
    rhs = [V*w | w], K identity-lhsT matmuls accumulate [128, F+H] in PSUM
    (weighted segment-sum + denominator), normalize, bias(+ReLU).
  * Next-layer tables h'=relu(o)@W', als'=o@(W'·a) are produced per window
    (PE transpose + one matmul) and AllGather'd across cores between layers.
  * segment_max is omitted: logits are bounded (|logit| < ~8), exp is safe
    in f32, and softmax is shift-invariant, so this is mathematically
    identical to the reference.
"""
import numpy as np

# problem constants (hardcoded per harness contract)
N, E, IN, HID, HEADS, OUT = 50000, 800000, 256, 32, 4, 64
SLOPE = 0.2
CORES = 8
P = 128


# ----------------------------------------------------------------------------
# host-side schedule construction (index data only)
# ----------------------------------------------------------------------------
def build_schedule(src, dst, ew, n_nodes, npad, cores):
    """Returns node permutation + per-core slot arrays.

    perm: old->new node id (len npad); Ks: [W] slots per window;
    gidx: [cores, 128, S] int32 src new-ids; logew: [cores, 128, S] f32.
    """
    nc_rows = npad // cores
    wpc = nc_rows // P
    src = np.asarray(src, np.int64)
    dst = np.asarray(dst, np.int64)
    ew = np.asarray(ew, np.float32)

    deg = np.bincount(dst, minlength=npad)
    order = np.argsort(-deg, kind="stable")          # ranks -> old id
    perm = np.empty(npad, np.int64)
    ranks = np.arange(npad)
    perm[order] = (ranks % cores) * nc_rows + ranks // cores

    nsrc = perm[src]
    ndst = perm[dst]
    eorder = np.argsort(ndst, kind="stable")
    nsrc_s = nsrc[eorder].astype(np.int32)
    ndst_s = ndst[eorder]
    ew_s = ew[eorder]

    counts = np.bincount(ndst_s, minlength=npad)
    starts = np.zeros(npad + 1, np.int64)
    np.cumsum(counts, out=starts[1:])
    rank_in_dst = np.arange(len(ndst_s)) - starts[ndst_s]

    Ks = counts.reshape(cores, wpc, P).max(axis=(0, 2))
    Ks = np.maximum(Ks, 1).astype(np.int64)
    offs = np.zeros(wpc + 1, np.int64)
    np.cumsum(Ks, out=offs[1:])
    S = int(offs[-1])

    core_e = ndst_s // nc_rows
    loc = ndst_s % nc_rows
    w_e = loc // P
    p_e = loc % P
    col = offs[w_e] + rank_in_dst

    gidx = np.zeros((cores, P, S), np.int32)
    logew = np.full((cores, P, S), -1e30, np.float32)
    flat = (core_e * P + p_e) * S + col
    gidx.reshape(-1)[flat] = nsrc_s
    with np.errstate(divide="ignore"):
        logew.reshape(-1)[flat] = np.log(np.maximum(ew_s, 0.0)).astype(np.float32)
    return perm, Ks, offs, gidx, logew


def _np_bf16(x):
    import ml_dtypes
    return np.asarray(x, np.float32).astype(ml_dtypes.bfloat16)


# ----------------------------------------------------------------------------
# device program
# ----------------------------------------------------------------------------
def build_program(npad, Ks, offs, S, in_f, hid_heads, out_f, heads3, cores,
                  enable_asserts=False, debug_taps=False):
    """Build the SPMD Bacc program. Shapes:
      xTt   [NT, in_f, 128] bf16   (transposed x, node-tile blocks)
      w1cat [in_f//128, 128, hid_heads+8] bf16
      w2cat [hid_heads, hid_heads+8] bf16
      w3cat [hid_heads, out_f+2] bf16
      b1row/b2row [1, hid_heads] f32 ; b3row [1, out_f] f32
      idxv  [128, S] int32 ; logew [128, S] f32 ; idx_ald [128, WPC] int32
      out   [NC, out_f] f32 (per-core shard)
    """
    import concourse.bacc as bacc
    import concourse.bass as bass
    import concourse.mybir as mybir
    from concourse.masks import make_identity
    from concourse.tile import TileContext

    F32, BF16, I32 = mybir.dt.float32, mybir.dt.bfloat16, mybir.dt.int32
    AF = mybir.ActivationFunctionType
    ALU = mybir.AluOpType

    nc_rows = npad // cores
    wpc = nc_rows // P
    nt = npad // P
    ic = in_f // P                       # input chunk count (2)
    D = hid_heads                         # 128
    H = HEADS
    wmax = int(max(Ks))

    nc = bacc.Bacc("TRN2", target_bir_lowering=False, debug=False,
                   enable_asserts=enable_asserts, num_devices=cores)

    xTt = nc.dram_tensor("xTt", [nt, in_f, P], BF16, kind="ExternalInput")
    w1cat = nc.dram_tensor("w1cat", [ic, P, D + 8], BF16, kind="ExternalInput")
    w2cat = nc.dram_tensor("w2cat", [D, D + 8], BF16, kind="ExternalInput")
    w3cat = nc.dram_tensor("w3cat", [D, out_f + 8], BF16, kind="ExternalInput")
    b1row = nc.dram_tensor("b1row", [1, D], F32, kind="ExternalInput")
    b2row = nc.dram_tensor("b2row", [1, D], F32, kind="ExternalInput")
    b3row = nc.dram_tensor("b3row", [1, out_f], F32, kind="ExternalInput")
    idxv = nc.dram_tensor("idxv", [P, S], I32, kind="ExternalInput")
    logew_d = nc.dram_tensor("logew", [P, S], F32, kind="ExternalInput")
    idx_ald = nc.dram_tensor("idx_ald", [P, wpc], I32, kind="ExternalInput")
    out_d = nc.dram_tensor("out", [nc_rows, out_f], F32, kind="ExternalOutput")
    dbg = {}
    if debug_taps:
        K0 = int(Ks[0])
        for nm, shp, dt in [
                ("dbg_h1", [P, D + H], F32),
                ("dbg_ald1", [P, H], F32),
                ("dbg_V", [P, K0, D + H], F32),
                ("dbg_wt", [P, K0, H], F32), ("dbg_acc", [P, D + H], F32),
                ("dbg_o", [P, D], F32), ("dbg_h2", [P, D + H], F32)]:
            dbg[nm] = nc.dram_tensor(nm, shp, dt, kind="ExternalOutput")

    # internal DRAM
    h1tab = nc.dram_tensor("h1tab", [npad, D + H], F32)
    ald1tab = nc.dram_tensor("ald1tab", [npad, H], F32)
    ag2h_in = nc.dram_tensor("ag2h_in", [nc_rows, D + H], F32)
    ald2sh = nc.dram_tensor("ald2sh", [nc_rows, H], F32)
    h2tab = nc.dram_tensor("h2tab", [npad, D + H], F32, addr_space="Shared")
    ag3h_in = nc.dram_tensor("ag3h_in", [nc_rows, out_f + heads3], F32)
    ald3sh = nc.dram_tensor("ald3sh", [nc_rows, heads3], F32)
    h3tab = nc.dram_tensor("h3tab", [npad, out_f + heads3], F32,
                           addr_space="Shared")

    rg = [list(range(cores))]

    with TileContext(nc) as tc:
        with tc.tile_pool(name="const", bufs=1) as cp, \
             tc.tile_pool(name="xin", bufs=3) as xp, \
             tc.tile_pool(name="work", bufs=2) as wk, \
             tc.tile_pool(name="small", bufs=3) as sm, \
             tc.tile_pool(name="ps", bufs=2, space="PSUM") as pp, \
             tc.tile_pool(name="ps2", bufs=2, space="PSUM") as pp2:

            ident_b = cp.tile([P, P], BF16, tag="identb")
            make_identity(nc, ident_b[:])
            ident_f = cp.tile([P, P], F32, tag="identf")
            make_identity(nc, ident_f[:])

            w1_sb = cp.tile([P, ic, D + 8], BF16, tag="w1")
            nc.sync.dma_start(out=w1_sb[:],
                              in_=w1cat.ap().rearrange("c p f -> p c f"))
            w2_sb = cp.tile([P, D + 8], BF16, tag="w2")
            nc.sync.dma_start(out=w2_sb[:], in_=w2cat[:, :])
            w3_sb = cp.tile([P, out_f + 8], BF16, tag="w3")
            nc.sync.dma_start(out=w3_sb[:], in_=w3cat[:, :])
            b1_sb = cp.tile([P, D], F32, tag="b1")
            nc.sync.dma_start(out=b1_sb[:], in_=b1row.ap().to_broadcast((P, D)))
            b2_sb = cp.tile([P, D], F32, tag="b2")
            nc.sync.dma_start(out=b2_sb[:], in_=b2row.ap().to_broadcast((P, D)))
            b3_sb = cp.tile([P, out_f], F32, tag="b3")
            nc.sync.dma_start(out=b3_sb[:],
                              in_=b3row.ap().to_broadcast((P, out_f)))
            idx_sb = cp.tile([P, S], I32, tag="idx")
            nc.sync.dma_start(out=idx_sb[:], in_=idxv[:, :])
            lew_sb = cp.tile([P, S], F32, tag="lew")
            nc.sync.dma_start(out=lew_sb[:], in_=logew_d[:, :])
            idxa_sb = cp.tile([P, wpc], I32, tag="idxa")
            nc.sync.dma_start(out=idxa_sb[:], in_=idx_ald[:, :])

            # ---------------- stage A: layer-1 tables (full, local) --------
            for t in range(nt):
                xt = xp.tile([P, ic, P], BF16, tag="xt")
                nc.sync.dma_start(
                    out=xt[:],
                    in_=xTt[t].rearrange("(c p) n -> p c n", p=P))
                ps = pp.tile([P, D + 8], F32, tag="psA")
                for c in range(ic):
                    nc.tensor.matmul(ps[:], lhsT=xt[:, c, :],
                                     rhs=w1_sb[:, c, :],
                                     start=(c == 0), stop=(c == ic - 1))
                h_sb = sm.tile([P, D + H], F32, tag="hA")
                nc.scalar.activation(h_sb[:], ps[:, 0:D + H], AF.Copy)
                nc.sync.dma_start(out=h1tab[t * P:(t + 1) * P, :], in_=h_sb[:])
                a_sb = sm.tile([P, H], F32, tag="aA")
                nc.vector.tensor_copy(out=a_sb[:], in_=ps[:, D + 4:D + 4 + H])
                nc.sync.dma_start(out=ald1tab[t * P:(t + 1) * P, :],
                                  in_=a_sb[:])

            # gather own ald windows into SBUF (core-dependent rows via data)
            ald1_all = cp.tile([P, wpc, H], F32, tag="ald1all")
            for w in range(wpc):
                nc.gpsimd.indirect_dma_start(
                    out=ald1_all[:, w, :], out_offset=None, in_=ald1tab[:, :],
                    in_offset=bass.IndirectOffsetOnAxis(
                        ap=idxa_sb[:, w:w + 1], axis=0))

            # ---------------- edge pass ------------------------------------
            def edge_layer(layer, tab, ald_src, F_in, H_l, wnext_sb,
                           F_next, H_n, bias_sb, agh, aldnext):
                """One GAT layer over all windows.
                ald_src: ("sbuf", tile) or ("dram", tensor)."""
                for w in range(wpc):
                    K = int(Ks[w])
                    off = int(offs[w])
                    r0 = w * P
                    V = wk.tile([P, K, F_in + H_l], F32, tag="V")
                    for k in range(K):
                        nc.gpsimd.indirect_dma_start(
                            out=V[:, k, :], out_offset=None, in_=tab[:, :],
                            in_offset=bass.IndirectOffsetOnAxis(
                                ap=idx_sb[:, off + k:off + k + 1], axis=0))
                    ALS = V[:, :, F_in:F_in + H_l]
                    if ald_src[0] == "sbuf":
                        ald_w = ald_src[1][:, w, :]
                    else:
                        ald_t = sm.tile([P, H_l], F32, tag="aldw")
                        nc.sync.dma_start(out=ald_t[:],
                                          in_=ald_src[1][r0:r0 + P, :])
                        ald_w = ald_t[:]
                    # logit = ALS + ald (bcast over K) + logew (bcast over H)
                    logit = wk.tile([P, K, H_l], F32, tag="logit")
                    ald_b = bass.AP(ald_w.tensor, ald_w.offset,
                                    [ald_w.ap[0], [0, K], [1, H_l]])
                    nc.vector.tensor_add(out=logit[:], in0=ALS, in1=ald_b)
                    lew_ap = lew_sb[:, off:off + K]
                    lew_b = bass.AP(lew_ap.tensor, lew_ap.offset,
                                    [lew_ap.ap[0], [1, K], [0, H_l]])
                    nc.vector.tensor_add(out=logit[:], in0=logit[:],
                                         in1=lew_b)
                    # w = exp(max(0.2*logit, logit))
                    wt = wk.tile([P, K, H_l], F32, tag="wt")
                    nc.vector.scalar_tensor_tensor(
                        out=wt[:], in0=logit[:], scalar=SLOPE, in1=logit[:],
                        op0=ALU.mult, op1=ALU.max)
                    nc.scalar.activation(wt[:], wt[:], AF.Exp)
                    if debug_taps and layer == 1 and w == 0:
                        nc.sync.dma_start(out=dbg["dbg_V"][:, :, :], in_=V[:])
                        nc.sync.dma_start(out=dbg["dbg_wt"][:, :, :],
                                          in_=wt[:])
                    wtb = wk.tile([P, K, H_l], BF16, tag="wtb")
                    nc.vector.tensor_copy(out=wtb[:], in_=wt[:])
                    # rhs = [V*w | w]
                    rhs = wk.tile([P, K, F_in + H_l], BF16, tag="rhs")
                    ch = F_in // H_l
                    wrep = bass.AP(wtb.tensor, wtb[:].offset,
                                   [wtb[:].ap[0], [H_l, K], [1, H_l], [0, ch]])
                    nc.vector.tensor_mul(out=rhs[:, :, 0:F_in],
                                         in0=V[:, :, 0:F_in], in1=wrep)
                    nc.vector.tensor_copy(out=rhs[:, :, F_in:F_in + H_l],
                                          in_=wtb[:])
                    acc = pp.tile([P, F_in + H_l], F32, tag="acc")
                    for k in range(K):
                        nc.tensor.matmul(acc[:], lhsT=ident_b[:],
                                         rhs=rhs[:, k, :],
                                         start=(k == 0), stop=(k == K - 1))
                    if debug_taps and layer == 1 and w == 0:
                        acc_dbg = sm.tile([P, F_in + H_l], F32, tag="accdbg")
                        nc.scalar.activation(acc_dbg[:], acc[:], AF.Copy)
                        nc.sync.dma_start(out=dbg["dbg_acc"][:, :],
                                          in_=acc_dbg[:])
                    den = sm.tile([P, H_l], F32, tag="den")
                    nc.vector.tensor_scalar_add(den[:],
                                                acc[:, F_in:F_in + H_l],
                                                1e-16)
                    rec = sm.tile([P, H_l], F32, tag="rec")
                    nc.vector.reciprocal(rec[:], den[:])
                    o = sm.tile([P, F_in], F32, tag="o")
                    rrep = bass.AP(rec.tensor, rec[:].offset,
                                   [rec[:].ap[0], [1, H_l], [0, ch]])
                    nc.vector.tensor_mul(out=o[:], in0=acc[:, 0:F_in],
                                         in1=rrep)
                    nc.vector.tensor_add(out=o[:], in0=o[:], in1=bias_sb[:])
                    if debug_taps and layer == 1 and w == 0:
                        nc.sync.dma_start(out=dbg["dbg_o"][:, :], in_=o[:])
                    if layer < 3:
                        nc.scalar.activation(o[:], o[:], AF.Relu)
                        # next-layer table rows for this window
                        oT = pp2.tile([P, P], F32, tag="oT")
                        nc.tensor.transpose(out=oT[:], in_=o[:],
                                            identity=ident_f[:])
                        oT_sb = sm.tile([P, P], BF16, tag="oTsb")
                        nc.scalar.activation(oT_sb[:], oT[:], AF.Copy)
                        hn = pp2.tile([P, F_next + 8], F32, tag="hn")
                        nc.tensor.matmul(hn[:], lhsT=oT_sb[:],
                                         rhs=wnext_sb[:, 0:F_next + 8],
                                         start=True, stop=True)
                        hn_sb = sm.tile([P, F_next + H_n], F32, tag="hnsb")
                        nc.scalar.activation(hn_sb[:, 0:F_next],
                                             hn[:, 0:F_next], AF.Copy)
                        nc.vector.tensor_copy(
                            out=hn_sb[:, F_next:F_next + H_n],
                            in_=hn[:, F_next:F_next + H_n])
                        nc.sync.dma_start(out=agh[r0:r0 + P, :], in_=hn_sb[:])
                        an_sb = sm.tile([P, H_n], F32, tag="ansb")
                        nc.vector.tensor_copy(
                            out=an_sb[:],
                            in_=hn[:, F_next + 4:F_next + 4 + H_n])
                        nc.sync.dma_start(out=aldnext[r0:r0 + P, :],
                                          in_=an_sb[:])
                    else:
                        nc.sync.dma_start(out=out_d[r0:r0 + P, :], in_=o[:])

            if debug_taps:
                nc.sync.dma_start(out=dbg["dbg_h1"][:, :],
                                  in_=h1tab[0:P, 0:D + H])
                nc.sync.dma_start(out=dbg["dbg_ald1"][:, :],
                                  in_=ald1tab[0:P, :])
            # layer 1
            edge_layer(1, h1tab, ("sbuf", ald1_all), D, H,
                       w2_sb, D, H, b1_sb, ag2h_in, ald2sh)
            nc.gpsimd.collective_compute(
                "AllGather", mybir.AluOpType.bypass, replica_groups=rg,
                ins=[ag2h_in.ap().opt()], outs=[h2tab.ap().opt()])
            if debug_taps:
                nc.sync.dma_start(out=dbg["dbg_h2"][:, :],
                                  in_=h2tab[0:P, 0:D + H])
            # layer 2
            edge_layer(2, h2tab, ("dram", ald2sh), D, H,
                       w3_sb, out_f, heads3, b2_sb, ag3h_in, ald3sh)
            nc.gpsimd.collective_compute(
                "AllGather", mybir.AluOpType.bypass, replica_groups=rg,
                ins=[ag3h_in.ap().opt()], outs=[h3tab.ap().opt()])
            # layer 3
            edge_layer(3, h3tab, ("dram", ald3sh), out_f, heads3,
                       None, 0, 1, b3_sb, None, None)
    nc.finalize()
    return nc


# ----------------------------------------------------------------------------
# host entry point
# ----------------------------------------------------------------------------
def prepare_inputs(x, edge_index, edge_weight, W1, a_src1, a_dst1, b1,
                   W2, a_src2, a_dst2, b2, W3, a_src3, a_dst3, b3,
                   npad, cores):
    """Returns (in_maps, perm, Ks, offs, S)."""
    x = np.asarray(x, np.float32)
    W1 = np.asarray(W1, np.float32)
    W2 = np.asarray(W2, np.float32)
    W3 = np.asarray(W3, np.float32)
    n_nodes, in_f = x.shape
    d1 = W1.shape[1]
    out_f = W3.shape[1]
    heads = np.asarray(a_src1).shape[0]
    hid = d1 // heads

    perm, Ks, offs, gidx, logew = build_schedule(
        edge_index[0], edge_index[1], edge_weight, n_nodes, npad, cores)

    xp = np.zeros((npad, in_f), np.float32)
    xp[perm[:n_nodes]] = x
    xTt = _np_bf16(xp.T.reshape(in_f, npad // P, P).transpose(1, 0, 2))

    def wcat(W, a_s, a_d, h, c):
        wa = (W.reshape(W.shape[0], h, c) * np.asarray(a_s)[None]).sum(-1)
        wd = (W.reshape(W.shape[0], h, c) * np.asarray(a_d)[None]).sum(-1)
        pad = np.zeros((W.shape[0], 4 - wa.shape[1]), np.float32)
        return np.concatenate([W, wa, pad, wd, pad], axis=1)

    w1full = wcat(W1, a_src1, a_dst1, heads, hid)          # [256, 136]
    w1cat = _np_bf16(w1full.reshape(2, P, d1 + 8))
    w2cat = _np_bf16(wcat(W2, a_src2, a_dst2, heads, hid))  # [128, 136]
    w3cat = _np_bf16(wcat(W3, a_src3, a_dst3, 1, out_f))    # [128, 72]

    nc_rows = npad // cores
    wpc = nc_rows // P
    in_maps = []
    for c in range(cores):
        base = c * nc_rows
        ia = (base + np.arange(wpc)[None, :] * P +
              np.arange(P)[:, None]).astype(np.int32)
        in_maps.append(dict(
            xTt=xTt, w1cat=w1cat, w2cat=w2cat, w3cat=w3cat,
            b1row=np.asarray(b1, np.float32).reshape(1, -1),
            b2row=np.asarray(b2, np.float32).reshape(1, -1),
            b3row=np.asarray(b3, np.float32).reshape(1, -1),
            idxv=gidx[c], logew=logew[c], idx_ald=ia,
        ))
    return in_maps, perm, Ks, offs


def kernel(**inputs):
    npad = 50176
    in_maps, perm, Ks, offs = prepare_inputs(
        npad=npad, cores=CORES, **inputs)
    S = int(offs[-1])
    nc = build_program(npad, Ks, offs, S, IN, HEADS * HID, OUT, 1, CORES)

    from concourse.bass_utils import run_bass_kernel_spmd
    res = run_bass_kernel_spmd(nc, in_maps, core_ids=list(range(CORES)))
    shards = [res.results[c]["out"] for c in range(CORES)]
    full = np.concatenate(shards, axis=0)       # [npad, OUT] in new-id order
    return full[perm[:N]].astype(np.float32)


# revision 27
# speedup vs baseline: 1.0079x; 1.0079x over previous
"""3-layer GAT on 8 TRN2 NeuronCores.

Strategy (1D vertex-cut, dst-sharded):
  * Nodes are permuted: degree-sorted, dealt round-robin to 8 cores, so each
    core owns a contiguous range of NC=6272 "new" node ids whose windows of
    128 consecutive ids have near-uniform in-degree.
  * Per (core, window) the edge list is laid out as K[w] "slots" x 128 dst
    rows (shared K schedule across cores, padding slots have log_ew=-1e30).
  * Per layer, per window: indirect-DMA gather of h[src] rows (bf16) and
    als[src] (f32), attention weight w = exp(leaky(als+ald)+log_ew),
    rhs = [V*w | w], K identity-lhsT matmuls accumulate [128, F+H] in PSUM
    (weighted segment-sum + denominator), normalize, bias(+ReLU).
  * Next-layer tables h'=relu(o)@W', als'=o@(W'·a) are produced per window
    (PE transpose + one matmul) and AllGather'd across cores between layers.
  * segment_max is omitted: logits are bounded (|logit| < ~8), exp is safe
    in f32, and softmax is shift-invariant, so this is mathematically
    identical to the reference.
"""
import numpy as np

# problem constants (hardcoded per harness contract)
N, E, IN, HID, HEADS, OUT = 50000, 800000, 256, 32, 4, 64
SLOPE = 0.2
CORES = 8
P = 128


# ----------------------------------------------------------------------------
# host-side schedule construction (index data only)
# ----------------------------------------------------------------------------
def build_schedule(src, dst, ew, n_nodes, npad, cores):
    """Returns node permutation + per-core slot arrays.

    perm: old->new node id (len npad); Ks: [W] slots per window;
    gidx: [cores, 128, S] int32 src new-ids; logew: [cores, 128, S] f32.
    """
    nc_rows = npad // cores
    wpc = nc_rows // P
    src = np.asarray(src, np.int64)
    dst = np.asarray(dst, np.int64)
    ew = np.asarray(ew, np.float32)

    deg = np.bincount(dst, minlength=npad)
    order = np.argsort(-deg, kind="stable")          # ranks -> old id
    perm = np.empty(npad, np.int64)
    ranks = np.arange(npad)
    perm[order] = (ranks % cores) * nc_rows + ranks // cores

    nsrc = perm[src]
    ndst = perm[dst]
    eorder = np.argsort(ndst, kind="stable")
    nsrc_s = nsrc[eorder].astype(np.int32)
    ndst_s = ndst[eorder]
    ew_s = ew[eorder]

    counts = np.bincount(ndst_s, minlength=npad)
    starts = np.zeros(npad + 1, np.int64)
    np.cumsum(counts, out=starts[1:])
    rank_in_dst = np.arange(len(ndst_s)) - starts[ndst_s]

    Ks = counts.reshape(cores, wpc, P).max(axis=(0, 2))
    Ks = np.maximum(Ks, 1).astype(np.int64)
    offs = np.zeros(wpc + 1, np.int64)
    np.cumsum(Ks, out=offs[1:])
    S = int(offs[-1])

    core_e = ndst_s // nc_rows
    loc = ndst_s % nc_rows
    w_e = loc // P
    p_e = loc % P
    col = offs[w_e] + rank_in_dst

    gidx = np.zeros((cores, P, S), np.int32)
    logew = np.full((cores, P, S), -1e30, np.float32)
    flat = (core_e * P + p_e) * S + col
    gidx.reshape(-1)[flat] = nsrc_s
    with np.errstate(divide="ignore"):
        logew.reshape(-1)[flat] = np.log(np.maximum(ew_s, 0.0)).astype(np.float32)
    return perm, Ks, offs, gidx, logew


def _np_bf16(x):
    import ml_dtypes
    return np.asarray(x, np.float32).astype(ml_dtypes.bfloat16)


# ----------------------------------------------------------------------------
# device program
# ----------------------------------------------------------------------------
def build_program(npad, Ks, offs, S, in_f, hid_heads, out_f, heads3, cores,
                  enable_asserts=False, debug_taps=False):
    """Build the SPMD Bacc program. Shapes:
      xTt   [NT, in_f, 128] bf16   (transposed x, node-tile blocks)
      w1cat [in_f//128, 128, hid_heads+8] bf16
      w2cat [hid_heads, hid_heads+8] bf16
      w3cat [hid_heads, out_f+2] bf16
      b1row/b2row [1, hid_heads] f32 ; b3row [1, out_f] f32
      idxv  [128, S] int32 ; logew [128, S] f32 ; idx_ald [128, WPC] int32
      out   [NC, out_f] f32 (per-core shard)
    """
    import concourse.bacc as bacc
    import concourse.bass as bass
    import concourse.mybir as mybir
    from concourse.masks import make_identity
    from concourse.tile import TileContext

    F32, BF16, I32 = mybir.dt.float32, mybir.dt.bfloat16, mybir.dt.int32
    AF = mybir.ActivationFunctionType
    ALU = mybir.AluOpType

    nc_rows = npad // cores
    wpc = nc_rows // P
    nt = npad // P
    ic = in_f // P                       # input chunk count (2)
    D = hid_heads                         # 128
    H = HEADS
    wmax = int(max(Ks))

    nc = bacc.Bacc("TRN2", target_bir_lowering=False, debug=False,
                   enable_asserts=enable_asserts, num_devices=cores)

    xTt = nc.dram_tensor("xTt", [nt, in_f, P], BF16, kind="ExternalInput")
    w1cat = nc.dram_tensor("w1cat", [ic, P, D + 8], BF16, kind="ExternalInput")
    w2cat = nc.dram_tensor("w2cat", [D, D + 8], BF16, kind="ExternalInput")
    w3cat = nc.dram_tensor("w3cat", [D, out_f + 8], BF16, kind="ExternalInput")
    b1row = nc.dram_tensor("b1row", [1, D], F32, kind="ExternalInput")
    b2row = nc.dram_tensor("b2row", [1, D], F32, kind="ExternalInput")
    b3row = nc.dram_tensor("b3row", [1, out_f], F32, kind="ExternalInput")
    idxv = nc.dram_tensor("idxv", [P, S], I32, kind="ExternalInput")
    logew_d = nc.dram_tensor("logew", [P, S], F32, kind="ExternalInput")
    idx_ald = nc.dram_tensor("idx_ald", [P, wpc], I32, kind="ExternalInput")
    out_d = nc.dram_tensor("out", [nc_rows, out_f], F32, kind="ExternalOutput")
    dbg = {}
    if debug_taps:
        K0 = int(Ks[0])
        for nm, shp, dt in [
                ("dbg_h1", [P, D + H], F32),
                ("dbg_ald1", [P, H], F32),
                ("dbg_V", [P, K0, D + H], F32),
                ("dbg_wt", [P, K0, H], F32), ("dbg_acc", [P, D + H], F32),
                ("dbg_o", [P, D], F32), ("dbg_h2", [P, D + H], F32)]:
            dbg[nm] = nc.dram_tensor(nm, shp, dt, kind="ExternalOutput")

    # internal DRAM
    h1tab = nc.dram_tensor("h1tab", [npad, D + H], F32)
    ald1tab = nc.dram_tensor("ald1tab", [npad, H], F32)
    ag2h_in = nc.dram_tensor("ag2h_in", [nc_rows, D + H], F32)
    ald2sh = nc.dram_tensor("ald2sh", [nc_rows, H], F32)
    h2tab = nc.dram_tensor("h2tab", [npad, D + H], F32, addr_space="Shared")
    ag3h_in = nc.dram_tensor("ag3h_in", [nc_rows, out_f + heads3], F32)
    ald3sh = nc.dram_tensor("ald3sh", [nc_rows, heads3], F32)
    h3tab = nc.dram_tensor("h3tab", [npad, out_f + heads3], F32,
                           addr_space="Shared")

    rg = [list(range(cores))]

    with TileContext(nc) as tc:
        with tc.tile_pool(name="const", bufs=1) as cp, \
             tc.tile_pool(name="xin", bufs=3) as xp, \
             tc.tile_pool(name="work", bufs=3) as wk, \
             tc.tile_pool(name="small", bufs=3) as sm, \
             tc.tile_pool(name="ps", bufs=2, space="PSUM") as pp, \
             tc.tile_pool(name="ps2", bufs=2, space="PSUM") as pp2:

            ident_b = cp.tile([P, P], BF16, tag="identb")
            make_identity(nc, ident_b[:])
            ident_f = cp.tile([P, P], F32, tag="identf")
            make_identity(nc, ident_f[:])

            w1_sb = cp.tile([P, ic, D + 8], BF16, tag="w1")
            nc.sync.dma_start(out=w1_sb[:],
                              in_=w1cat.ap().rearrange("c p f -> p c f"))
            w2_sb = cp.tile([P, D + 8], BF16, tag="w2")
            nc.sync.dma_start(out=w2_sb[:], in_=w2cat[:, :])
            w3_sb = cp.tile([P, out_f + 8], BF16, tag="w3")
            nc.sync.dma_start(out=w3_sb[:], in_=w3cat[:, :])
            b1_sb = cp.tile([P, D], F32, tag="b1")
            nc.sync.dma_start(out=b1_sb[:], in_=b1row.ap().to_broadcast((P, D)))
            b2_sb = cp.tile([P, D], F32, tag="b2")
            nc.sync.dma_start(out=b2_sb[:], in_=b2row.ap().to_broadcast((P, D)))
            b3_sb = cp.tile([P, out_f], F32, tag="b3")
            nc.sync.dma_start(out=b3_sb[:],
                              in_=b3row.ap().to_broadcast((P, out_f)))
            idx_sb = cp.tile([P, S], I32, tag="idx")
            nc.sync.dma_start(out=idx_sb[:], in_=idxv[:, :])
            lew_sb = cp.tile([P, S], F32, tag="lew")
            nc.sync.dma_start(out=lew_sb[:], in_=logew_d[:, :])
            idxa_sb = cp.tile([P, wpc], I32, tag="idxa")
            nc.sync.dma_start(out=idxa_sb[:], in_=idx_ald[:, :])

            # ---------------- stage A: layer-1 tables (full, local) --------
            for t in range(nt):
                xt = xp.tile([P, ic, P], BF16, tag="xt")
                nc.sync.dma_start(
                    out=xt[:],
                    in_=xTt[t].rearrange("(c p) n -> p c n", p=P))
                ps = pp.tile([P, D + 8], F32, tag="psA")
                for c in range(ic):
                    nc.tensor.matmul(ps[:], lhsT=xt[:, c, :],
                                     rhs=w1_sb[:, c, :],
                                     start=(c == 0), stop=(c == ic - 1))
                h_sb = sm.tile([P, D + H], F32, tag="hA")
                nc.scalar.activation(h_sb[:], ps[:, 0:D + H], AF.Copy)
                nc.sync.dma_start(out=h1tab[t * P:(t + 1) * P, :], in_=h_sb[:])
                a_sb = sm.tile([P, H], F32, tag="aA")
                nc.vector.tensor_copy(out=a_sb[:], in_=ps[:, D + 4:D + 4 + H])
                nc.sync.dma_start(out=ald1tab[t * P:(t + 1) * P, :],
                                  in_=a_sb[:])

            # gather own ald windows into SBUF (core-dependent rows via data)
            ald1_all = cp.tile([P, wpc, H], F32, tag="ald1all")
            for w in range(wpc):
                nc.gpsimd.indirect_dma_start(
                    out=ald1_all[:, w, :], out_offset=None, in_=ald1tab[:, :],
                    in_offset=bass.IndirectOffsetOnAxis(
                        ap=idxa_sb[:, w:w + 1], axis=0))

            # ---------------- edge pass ------------------------------------
            def edge_layer(layer, tab, ald_src, F_in, H_l, wnext_sb,
                           F_next, H_n, bias_sb, agh, aldnext):
                """One GAT layer over all windows.
                ald_src: ("sbuf", tile) or ("dram", tensor)."""
                for w in range(wpc):
                    K = int(Ks[w])
                    off = int(offs[w])
                    r0 = w * P
                    V = wk.tile([P, K, F_in + H_l], F32, tag="V")
                    for k in range(K):
                        nc.gpsimd.indirect_dma_start(
                            out=V[:, k, :], out_offset=None, in_=tab[:, :],
                            in_offset=bass.IndirectOffsetOnAxis(
                                ap=idx_sb[:, off + k:off + k + 1], axis=0))
                    ALS = V[:, :, F_in:F_in + H_l]
                    if ald_src[0] == "sbuf":
                        ald_w = ald_src[1][:, w, :]
                    else:
                        ald_t = sm.tile([P, H_l], F32, tag="aldw")
                        nc.sync.dma_start(out=ald_t[:],
                                          in_=ald_src[1][r0:r0 + P, :])
                        ald_w = ald_t[:]
                    # logit = ALS + ald (bcast over K) + logew (bcast over H)
                    logit = wk.tile([P, K, H_l], F32, tag="logit")
                    ald_b = bass.AP(ald_w.tensor, ald_w.offset,
                                    [ald_w.ap[0], [0, K], [1, H_l]])
                    nc.vector.tensor_add(out=logit[:], in0=ALS, in1=ald_b)
                    lew_ap = lew_sb[:, off:off + K]
                    lew_b = bass.AP(lew_ap.tensor, lew_ap.offset,
                                    [lew_ap.ap[0], [1, K], [0, H_l]])
                    nc.vector.tensor_add(out=logit[:], in0=logit[:],
                                         in1=lew_b)
                    # w = exp(max(0.2*logit, logit))
                    wt = wk.tile([P, K, H_l], F32, tag="wt")
                    nc.vector.scalar_tensor_tensor(
                        out=wt[:], in0=logit[:], scalar=SLOPE, in1=logit[:],
                        op0=ALU.mult, op1=ALU.max)
                    nc.scalar.activation(wt[:], wt[:], AF.Exp)
                    if debug_taps and layer == 1 and w == 0:
                        nc.sync.dma_start(out=dbg["dbg_V"][:, :, :], in_=V[:])
                        nc.sync.dma_start(out=dbg["dbg_wt"][:, :, :],
                                          in_=wt[:])
                    wtb = wk.tile([P, K, H_l], BF16, tag="wtb")
                    nc.vector.tensor_copy(out=wtb[:], in_=wt[:])
                    # rhs = [V*w | w]
                    rhs = wk.tile([P, K, F_in + H_l], BF16, tag="rhs")
                    ch = F_in // H_l
                    wrep = bass.AP(wtb.tensor, wtb[:].offset,
                                   [wtb[:].ap[0], [H_l, K], [1, H_l], [0, ch]])
                    nc.vector.tensor_mul(out=rhs[:, :, 0:F_in],
                                         in0=V[:, :, 0:F_in], in1=wrep)
                    nc.vector.tensor_copy(out=rhs[:, :, F_in:F_in + H_l],
                                          in_=wtb[:])
                    acc = pp.tile([P, F_in + H_l], F32, tag="acc")
                    for k in range(K):
                        nc.tensor.matmul(acc[:], lhsT=ident_b[:],
                                         rhs=rhs[:, k, :],
                                         start=(k == 0), stop=(k == K - 1))
                    if debug_taps and layer == 1 and w == 0:
                        acc_dbg = sm.tile([P, F_in + H_l], F32, tag="accdbg")
                        nc.scalar.activation(acc_dbg[:], acc[:], AF.Copy)
                        nc.sync.dma_start(out=dbg["dbg_acc"][:, :],
                                          in_=acc_dbg[:])
                    den = sm.tile([P, H_l], F32, tag="den")
                    nc.vector.tensor_scalar_add(den[:],
                                                acc[:, F_in:F_in + H_l],
                                                1e-16)
                    rec = sm.tile([P, H_l], F32, tag="rec")
                    nc.vector.reciprocal(rec[:], den[:])
                    o = sm.tile([P, F_in], F32, tag="o")
                    rrep = bass.AP(rec.tensor, rec[:].offset,
                                   [rec[:].ap[0], [1, H_l], [0, ch]])
                    nc.vector.tensor_mul(out=o[:], in0=acc[:, 0:F_in],
                                         in1=rrep)
                    nc.vector.tensor_add(out=o[:], in0=o[:], in1=bias_sb[:])
                    if debug_taps and layer == 1 and w == 0:
                        nc.sync.dma_start(out=dbg["dbg_o"][:, :], in_=o[:])
                    if layer < 3:
                        nc.scalar.activation(o[:], o[:], AF.Relu)
                        # next-layer table rows for this window
                        oT = pp2.tile([P, P], F32, tag="oT")
                        nc.tensor.transpose(out=oT[:], in_=o[:],
                                            identity=ident_f[:])
                        oT_sb = sm.tile([P, P], BF16, tag="oTsb")
                        nc.scalar.activation(oT_sb[:], oT[:], AF.Copy)
                        hn = pp2.tile([P, F_next + 8], F32, tag="hn")
                        nc.tensor.matmul(hn[:], lhsT=oT_sb[:],
                                         rhs=wnext_sb[:, 0:F_next + 8],
                                         start=True, stop=True)
                        hn_sb = sm.tile([P, F_next + H_n], F32, tag="hnsb")
                        nc.scalar.activation(hn_sb[:, 0:F_next],
                                             hn[:, 0:F_next], AF.Copy)
                        nc.vector.tensor_copy(
                            out=hn_sb[:, F_next:F_next + H_n],
                            in_=hn[:, F_next:F_next + H_n])
                        nc.sync.dma_start(out=agh[r0:r0 + P, :], in_=hn_sb[:])
                        an_sb = sm.tile([P, H_n], F32, tag="ansb")
                        nc.vector.tensor_copy(
                            out=an_sb[:],
                            in_=hn[:, F_next + 4:F_next + 4 + H_n])
                        nc.sync.dma_start(out=aldnext[r0:r0 + P, :],
                                          in_=an_sb[:])
                    else:
                        nc.sync.dma_start(out=out_d[r0:r0 + P, :], in_=o[:])

            if debug_taps:
                nc.sync.dma_start(out=dbg["dbg_h1"][:, :],
                                  in_=h1tab[0:P, 0:D + H])
                nc.sync.dma_start(out=dbg["dbg_ald1"][:, :],
                                  in_=ald1tab[0:P, :])
            # layer 1
            edge_layer(1, h1tab, ("sbuf", ald1_all), D, H,
                       w2_sb, D, H, b1_sb, ag2h_in, ald2sh)
            nc.gpsimd.collective_compute(
                "AllGather", mybir.AluOpType.bypass, replica_groups=rg,
                ins=[ag2h_in.ap().opt()], outs=[h2tab.ap().opt()])
            if debug_taps:
                nc.sync.dma_start(out=dbg["dbg_h2"][:, :],
                                  in_=h2tab[0:P, 0:D + H])
            # layer 2
            edge_layer(2, h2tab, ("dram", ald2sh), D, H,
                       w3_sb, out_f, heads3, b2_sb, ag3h_in, ald3sh)
            nc.gpsimd.collective_compute(
                "AllGather", mybir.AluOpType.bypass, replica_groups=rg,
                ins=[ag3h_in.ap().opt()], outs=[h3tab.ap().opt()])
            # layer 3
            edge_layer(3, h3tab, ("dram", ald3sh), out_f, heads3,
                       None, 0, 1, b3_sb, None, None)
    nc.finalize()
    return nc


# ----------------------------------------------------------------------------
# host entry point
# ----------------------------------------------------------------------------
def prepare_inputs(x, edge_index, edge_weight, W1, a_src1, a_dst1, b1,
                   W2, a_src2, a_dst2, b2, W3, a_src3, a_dst3, b3,
                   npad, cores):
    """Returns (in_maps, perm, Ks, offs, S)."""
    x = np.asarray(x, np.float32)
    W1 = np.asarray(W1, np.float32)
    W2 = np.asarray(W2, np.float32)
    W3 = np.asarray(W3, np.float32)
    n_nodes, in_f = x.shape
    d1 = W1.shape[1]
    out_f = W3.shape[1]
    heads = np.asarray(a_src1).shape[0]
    hid = d1 // heads

    perm, Ks, offs, gidx, logew = build_schedule(
        edge_index[0], edge_index[1], edge_weight, n_nodes, npad, cores)

    xp = np.zeros((npad, in_f), np.float32)
    xp[perm[:n_nodes]] = x
    xTt = _np_bf16(xp.T.reshape(in_f, npad // P, P).transpose(1, 0, 2))

    def wcat(W, a_s, a_d, h, c):
        wa = (W.reshape(W.shape[0], h, c) * np.asarray(a_s)[None]).sum(-1)
        wd = (W.reshape(W.shape[0], h, c) * np.asarray(a_d)[None]).sum(-1)
        pad = np.zeros((W.shape[0], 4 - wa.shape[1]), np.float32)
        return np.concatenate([W, wa, pad, wd, pad], axis=1)

    w1full = wcat(W1, a_src1, a_dst1, heads, hid)          # [256, 136]
    w1cat = _np_bf16(w1full.reshape(2, P, d1 + 8))
    w2cat = _np_bf16(wcat(W2, a_src2, a_dst2, heads, hid))  # [128, 136]
    w3cat = _np_bf16(wcat(W3, a_src3, a_dst3, 1, out_f))    # [128, 72]

    nc_rows = npad // cores
    wpc = nc_rows // P
    in_maps = []
    for c in range(cores):
        base = c * nc_rows
        ia = (base + np.arange(wpc)[None, :] * P +
              np.arange(P)[:, None]).astype(np.int32)
        in_maps.append(dict(
            xTt=xTt, w1cat=w1cat, w2cat=w2cat, w3cat=w3cat,
            b1row=np.asarray(b1, np.float32).reshape(1, -1),
            b2row=np.asarray(b2, np.float32).reshape(1, -1),
            b3row=np.asarray(b3, np.float32).reshape(1, -1),
            idxv=gidx[c], logew=logew[c], idx_ald=ia,
        ))
    return in_maps, perm, Ks, offs


def kernel(**inputs):
    npad = 50176
    in_maps, perm, Ks, offs = prepare_inputs(
        npad=npad, cores=CORES, **inputs)
    S = int(offs[-1])
    nc = build_program(npad, Ks, offs, S, IN, HEADS * HID, OUT, 1, CORES)

    from concourse.bass_utils import run_bass_kernel_spmd
    res = run_bass_kernel_spmd(nc, in_maps, core_ids=list(range(CORES)))
    shards = [res.results[c]["out"] for c in range(CORES)]
    full = np.concatenate(shards, axis=0)       # [npad, OUT] in new-id order
    return full[perm[:N]].astype(np.float32)


# revision 29
# speedup vs baseline: 1.0207x; 1.0127x over previous
"""3-layer GAT on 8 TRN2 NeuronCores.

Strategy (1D vertex-cut, dst-sharded):
  * Nodes are permuted: degree-sorted, dealt round-robin to 8 cores, so each
    core owns a contiguous range of NC=6272 "new" node ids whose windows of
    128 consecutive ids have near-uniform in-degree.
  * Per (core, window) the edge list is laid out as K[w] "slots" x 128 dst
    rows (shared K schedule across cores, padding slots have log_ew=-1e30).
  * Per layer, per window: indirect-DMA gather of h[src] rows (bf16) and
    als[src] (f32), attention weight w = exp(leaky(als+ald)+log_ew),
    rhs = [V*w | w], K identity-lhsT matmuls accumulate [128, F+H] in PSUM
    (weighted segment-sum + denominator), normalize, bias(+ReLU).
  * Next-layer tables h'=relu(o)@W', als'=o@(W'·a) are produced per window
    (PE transpose + one matmul) and AllGather'd across cores between layers.
  * segment_max is omitted: logits are bounded (|logit| < ~8), exp is safe
    in f32, and softmax is shift-invariant, so this is mathematically
    identical to the reference.
"""
import numpy as np

# problem constants (hardcoded per harness contract)
N, E, IN, HID, HEADS, OUT = 50000, 800000, 256, 32, 4, 64
SLOPE = 0.2
CORES = 8
P = 128


# ----------------------------------------------------------------------------
# host-side schedule construction (index data only)
# ----------------------------------------------------------------------------
def build_schedule(src, dst, ew, n_nodes, npad, cores):
    """Returns node permutation + per-core slot arrays.

    perm: old->new node id (len npad); Ks: [W] slots per window;
    gidx: [cores, 128, S] int32 src new-ids; logew: [cores, 128, S] f32.
    """
    nc_rows = npad // cores
    wpc = nc_rows // P
    src = np.asarray(src, np.int64)
    dst = np.asarray(dst, np.int64)
    ew = np.asarray(ew, np.float32)

    deg = np.bincount(dst, minlength=npad)
    order = np.argsort(-deg, kind="stable")          # ranks -> old id
    perm = np.empty(npad, np.int64)
    ranks = np.arange(npad)
    perm[order] = (ranks % cores) * nc_rows + ranks // cores

    nsrc = perm[src]
    ndst = perm[dst]
    eorder = np.argsort(ndst, kind="stable")
    nsrc_s = nsrc[eorder].astype(np.int32)
    ndst_s = ndst[eorder]
    ew_s = ew[eorder]

    counts = np.bincount(ndst_s, minlength=npad)
    starts = np.zeros(npad + 1, np.int64)
    np.cumsum(counts, out=starts[1:])
    rank_in_dst = np.arange(len(ndst_s)) - starts[ndst_s]

    Ks = counts.reshape(cores, wpc, P).max(axis=(0, 2))
    Ks = np.maximum(Ks, 1).astype(np.int64)
    offs = np.zeros(wpc + 1, np.int64)
    np.cumsum(Ks, out=offs[1:])
    S = int(offs[-1])

    core_e = ndst_s // nc_rows
    loc = ndst_s % nc_rows
    w_e = loc // P
    p_e = loc % P
    col = offs[w_e] + rank_in_dst

    gidx = np.zeros((cores, P, S), np.int32)
    logew = np.full((cores, P, S), -1e30, np.float32)
    flat = (core_e * P + p_e) * S + col
    gidx.reshape(-1)[flat] = nsrc_s
    with np.errstate(divide="ignore"):
        logew.reshape(-1)[flat] = np.log(np.maximum(ew_s, 0.0)).astype(np.float32)
    return perm, Ks, offs, gidx, logew


def _np_bf16(x):
    import ml_dtypes
    return np.asarray(x, np.float32).astype(ml_dtypes.bfloat16)


# ----------------------------------------------------------------------------
# device program
# ----------------------------------------------------------------------------
def build_program(npad, Ks, offs, S, in_f, hid_heads, out_f, heads3, cores,
                  enable_asserts=False, debug_taps=False):
    """Build the SPMD Bacc program. Shapes:
      xTt   [NT, in_f, 128] bf16   (transposed x, node-tile blocks)
      w1cat [in_f//128, 128, hid_heads+8] bf16
      w2cat [hid_heads, hid_heads+8] bf16
      w3cat [hid_heads, out_f+2] bf16
      b1row/b2row [1, hid_heads] f32 ; b3row [1, out_f] f32
      idxv  [128, S] int32 ; logew [128, S] f32 ; idx_ald [128, WPC] int32
      out   [NC, out_f] f32 (per-core shard)
    """
    import concourse.bacc as bacc
    import concourse.bass as bass
    import concourse.mybir as mybir
    from concourse.masks import make_identity
    from concourse.tile import TileContext

    F32, BF16, I32 = mybir.dt.float32, mybir.dt.bfloat16, mybir.dt.int32
    AF = mybir.ActivationFunctionType
    ALU = mybir.AluOpType

    nc_rows = npad // cores
    wpc = nc_rows // P
    nt = npad // P
    ic = in_f // P                       # input chunk count (2)
    D = hid_heads                         # 128
    H = HEADS
    wmax = int(max(Ks))

    nc = bacc.Bacc("TRN2", target_bir_lowering=False, debug=False,
                   enable_asserts=enable_asserts, num_devices=cores)

    xTt = nc.dram_tensor("xTt", [nt, in_f, P], BF16, kind="ExternalInput")
    w1cat = nc.dram_tensor("w1cat", [ic, P, D + 8], BF16, kind="ExternalInput")
    w2cat = nc.dram_tensor("w2cat", [D, D + 8], BF16, kind="ExternalInput")
    w3cat = nc.dram_tensor("w3cat", [D, out_f + 8], BF16, kind="ExternalInput")
    b1row = nc.dram_tensor("b1row", [1, D], F32, kind="ExternalInput")
    b2row = nc.dram_tensor("b2row", [1, D], F32, kind="ExternalInput")
    b3row = nc.dram_tensor("b3row", [1, out_f], F32, kind="ExternalInput")
    idxv = nc.dram_tensor("idxv", [P, S], I32, kind="ExternalInput")
    logew_d = nc.dram_tensor("logew", [P, S], F32, kind="ExternalInput")
    idx_ald = nc.dram_tensor("idx_ald", [P, wpc], I32, kind="ExternalInput")
    out_d = nc.dram_tensor("out", [nc_rows, out_f], F32, kind="ExternalOutput")
    dbg = {}
    if debug_taps:
        K0 = int(Ks[0])
        for nm, shp, dt in [
                ("dbg_h1", [P, D + H], F32),
                ("dbg_ald1", [P, H], F32),
                ("dbg_V", [P, K0, D + H], F32),
                ("dbg_wt", [P, K0, H], F32), ("dbg_acc", [P, D + H], F32),
                ("dbg_o", [P, D], F32), ("dbg_h2", [P, D + H], F32)]:
            dbg[nm] = nc.dram_tensor(nm, shp, dt, kind="ExternalOutput")

    # internal DRAM
    h1tab = nc.dram_tensor("h1tab", [npad, D + H], F32)
    ald1tab = nc.dram_tensor("ald1tab", [npad, H], F32)
    ag2h_in = nc.dram_tensor("ag2h_in", [nc_rows, D + H], F32)
    ald2sh = nc.dram_tensor("ald2sh", [nc_rows, H], F32)
    h2tab = nc.dram_tensor("h2tab", [npad, D + H], F32, addr_space="Shared")
    ag3h_in = nc.dram_tensor("ag3h_in", [nc_rows, out_f + heads3], F32)
    ald3sh = nc.dram_tensor("ald3sh", [nc_rows, heads3], F32)
    h3tab = nc.dram_tensor("h3tab", [npad, out_f + heads3], F32,
                           addr_space="Shared")

    rg = [list(range(cores))]

    with TileContext(nc) as tc:
        with tc.tile_pool(name="const", bufs=1) as cp, \
             tc.tile_pool(name="xin", bufs=3) as xp, \
             tc.tile_pool(name="work", bufs=3) as wk, \
             tc.tile_pool(name="small", bufs=3) as sm, \
             tc.tile_pool(name="ps", bufs=2, space="PSUM") as pp, \
             tc.tile_pool(name="ps2", bufs=2, space="PSUM") as pp2:

            ident_b = cp.tile([P, P], BF16, tag="identb")
            make_identity(nc, ident_b[:])
            ident_f = cp.tile([P, P], F32, tag="identf")
            make_identity(nc, ident_f[:])

            w1_sb = cp.tile([P, ic, D + 8], BF16, tag="w1")
            nc.sync.dma_start(out=w1_sb[:],
                              in_=w1cat.ap().rearrange("c p f -> p c f"))
            w2_sb = cp.tile([P, D + 8], BF16, tag="w2")
            nc.sync.dma_start(out=w2_sb[:], in_=w2cat[:, :])
            w3_sb = cp.tile([P, out_f + 8], BF16, tag="w3")
            nc.sync.dma_start(out=w3_sb[:], in_=w3cat[:, :])
            b1_sb = cp.tile([P, D], F32, tag="b1")
            nc.sync.dma_start(out=b1_sb[:], in_=b1row.ap().to_broadcast((P, D)))
            b2_sb = cp.tile([P, D], F32, tag="b2")
            nc.sync.dma_start(out=b2_sb[:], in_=b2row.ap().to_broadcast((P, D)))
            b3_sb = cp.tile([P, out_f], F32, tag="b3")
            nc.sync.dma_start(out=b3_sb[:],
                              in_=b3row.ap().to_broadcast((P, out_f)))
            idx_sb = cp.tile([P, S], I32, tag="idx")
            nc.sync.dma_start(out=idx_sb[:], in_=idxv[:, :])
            lew_sb = cp.tile([P, S], F32, tag="lew")
            nc.sync.dma_start(out=lew_sb[:], in_=logew_d[:, :])
            idxa_sb = cp.tile([P, wpc], I32, tag="idxa")
            nc.sync.dma_start(out=idxa_sb[:], in_=idx_ald[:, :])

            # ---------------- stage A: layer-1 tables (full, local) --------
            for t in range(nt):
                xt = xp.tile([P, ic, P], BF16, tag="xt")
                nc.sync.dma_start(
                    out=xt[:],
                    in_=xTt[t].rearrange("(c p) n -> p c n", p=P))
                ps = pp.tile([P, D + 8], F32, tag="psA")
                for c in range(ic):
                    nc.tensor.matmul(ps[:], lhsT=xt[:, c, :],
                                     rhs=w1_sb[:, c, :],
                                     start=(c == 0), stop=(c == ic - 1))
                h_sb = sm.tile([P, D + H], F32, tag="hA")
                nc.scalar.activation(h_sb[:], ps[:, 0:D + H], AF.Copy)
                nc.sync.dma_start(out=h1tab[t * P:(t + 1) * P, :], in_=h_sb[:])
                a_sb = sm.tile([P, H], F32, tag="aA")
                nc.vector.tensor_copy(out=a_sb[:], in_=ps[:, D + 4:D + 4 + H])
                nc.sync.dma_start(out=ald1tab[t * P:(t + 1) * P, :],
                                  in_=a_sb[:])

            # gather own ald windows into SBUF (core-dependent rows via data)
            ald1_all = cp.tile([P, wpc, H], F32, tag="ald1all")
            for w in range(wpc):
                nc.gpsimd.indirect_dma_start(
                    out=ald1_all[:, w, :], out_offset=None, in_=ald1tab[:, :],
                    in_offset=bass.IndirectOffsetOnAxis(
                        ap=idxa_sb[:, w:w + 1], axis=0))

            # ---------------- edge pass ------------------------------------
            def edge_layer(layer, tab, ald_src, F_in, H_l, wnext_sb,
                           F_next, H_n, bias_sb, agh, aldnext):
                """One GAT layer over all windows.
                ald_src: ("sbuf", tile) or ("dram", tensor)."""
                for w in range(wpc):
                    K = int(Ks[w])
                    off = int(offs[w])
                    r0 = w * P
                    V = wk.tile([P, K, F_in + H_l], F32, tag="V")
                    for k in range(K):
                        nc.gpsimd.indirect_dma_start(
                            out=V[:, k, :], out_offset=None, in_=tab[:, :],
                            in_offset=bass.IndirectOffsetOnAxis(
                                ap=idx_sb[:, off + k:off + k + 1], axis=0))
                    ALS = V[:, :, F_in:F_in + H_l]
                    if ald_src[0] == "sbuf":
                        ald_w = ald_src[1][:, w, :]
                    else:
                        ald_t = sm.tile([P, H_l], F32, tag="aldw")
                        nc.sync.dma_start(out=ald_t[:],
                                          in_=ald_src[1][r0:r0 + P, :])
                        ald_w = ald_t[:]
                    # logit = ALS + ald (bcast over K) + logew (bcast over H)
                    logit = wk.tile([P, K, H_l], F32, tag="logit")
                    ald_b = bass.AP(ald_w.tensor, ald_w.offset,
                                    [ald_w.ap[0], [0, K], [1, H_l]])
                    nc.vector.tensor_add(out=logit[:], in0=ALS, in1=ald_b)
                    lew_ap = lew_sb[:, off:off + K]
                    lew_b = bass.AP(lew_ap.tensor, lew_ap.offset,
                                    [lew_ap.ap[0], [1, K], [0, H_l]])
                    nc.vector.tensor_add(out=logit[:], in0=logit[:],
                                         in1=lew_b)
                    # w = exp(max(0.2*logit, logit))
                    wt = wk.tile([P, K, H_l], F32, tag="wt")
                    nc.vector.scalar_tensor_tensor(
                        out=wt[:], in0=logit[:], scalar=SLOPE, in1=logit[:],
                        op0=ALU.mult, op1=ALU.max)
                    nc.scalar.activation(wt[:], wt[:], AF.Exp)
                    if debug_taps and layer == 1 and w == 0:
                        nc.sync.dma_start(out=dbg["dbg_V"][:, :, :], in_=V[:])
                        nc.sync.dma_start(out=dbg["dbg_wt"][:, :, :],
                                          in_=wt[:])
                    wtb = wk.tile([P, K, H_l], BF16, tag="wtb")
                    nc.vector.tensor_copy(out=wtb[:], in_=wt[:])
                    # rhs = [V*w | w]
                    rhs = wk.tile([P, K, F_in + H_l], BF16, tag="rhs")
                    ch = F_in // H_l
                    wrep = bass.AP(wtb.tensor, wtb[:].offset,
                                   [wtb[:].ap[0], [H_l, K], [1, H_l], [0, ch]])
                    nc.vector.tensor_mul(out=rhs[:, :, 0:F_in],
                                         in0=V[:, :, 0:F_in], in1=wrep)
                    nc.vector.tensor_copy(out=rhs[:, :, F_in:F_in + H_l],
                                          in_=wtb[:])
                    acc = pp.tile([P, F_in + H_l], F32, tag="acc")
                    for k in range(K):
                        nc.tensor.matmul(acc[:], lhsT=ident_b[:],
                                         rhs=rhs[:, k, :],
                                         start=(k == 0), stop=(k == K - 1))
                    if debug_taps and layer == 1 and w == 0:
                        acc_dbg = sm.tile([P, F_in + H_l], F32, tag="accdbg")
                        nc.scalar.activation(acc_dbg[:], acc[:], AF.Copy)
                        nc.sync.dma_start(out=dbg["dbg_acc"][:, :],
                                          in_=acc_dbg[:])
                    den = sm.tile([P, H_l], F32, tag="den")
                    nc.vector.tensor_scalar_add(den[:],
                                                acc[:, F_in:F_in + H_l],
                                                1e-16)
                    rec = sm.tile([P, H_l], F32, tag="rec")
                    nc.vector.reciprocal(rec[:], den[:])
                    o = sm.tile([P, F_in], F32, tag="o")
                    rrep = bass.AP(rec.tensor, rec[:].offset,
                                   [rec[:].ap[0], [1, H_l], [0, ch]])
                    nc.vector.tensor_mul(out=o[:], in0=acc[:, 0:F_in],
                                         in1=rrep)
                    nc.vector.tensor_add(out=o[:], in0=o[:], in1=bias_sb[:])
                    if debug_taps and layer == 1 and w == 0:
                        nc.sync.dma_start(out=dbg["dbg_o"][:, :], in_=o[:])
                    if layer < 3:
                        nc.scalar.activation(o[:], o[:], AF.Relu)
                        # next-layer table rows for this window
                        oT = pp2.tile([P, P], F32, tag="oT")
                        nc.tensor.transpose(out=oT[:], in_=o[:],
                                            identity=ident_f[:])
                        oT_sb = sm.tile([P, P], BF16, tag="oTsb")
                        nc.scalar.activation(oT_sb[:], oT[:], AF.Copy)
                        hn = pp2.tile([P, F_next + 8], F32, tag="hn")
                        nc.tensor.matmul(hn[:], lhsT=oT_sb[:],
                                         rhs=wnext_sb[:, 0:F_next + 8],
                                         start=True, stop=True)
                        hn_sb = sm.tile([P, F_next + H_n], F32, tag="hnsb")
                        nc.scalar.activation(hn_sb[:, 0:F_next],
                                             hn[:, 0:F_next], AF.Copy)
                        nc.vector.tensor_copy(
                            out=hn_sb[:, F_next:F_next + H_n],
                            in_=hn[:, F_next:F_next + H_n])
                        nc.sync.dma_start(out=agh[r0:r0 + P, :], in_=hn_sb[:])
                        an_sb = sm.tile([P, H_n], F32, tag="ansb")
                        nc.vector.tensor_copy(
                            out=an_sb[:],
                            in_=hn[:, F_next + 4:F_next + 4 + H_n])
                        nc.sync.dma_start(out=aldnext[r0:r0 + P, :],
                                          in_=an_sb[:])
                    else:
                        nc.sync.dma_start(out=out_d[r0:r0 + P, :], in_=o[:])

            if debug_taps:
                nc.sync.dma_start(out=dbg["dbg_h1"][:, :],
                                  in_=h1tab[0:P, 0:D + H])
                nc.sync.dma_start(out=dbg["dbg_ald1"][:, :],
                                  in_=ald1tab[0:P, :])
            # layer 1
            edge_layer(1, h1tab, ("sbuf", ald1_all), D, H,
                       w2_sb, D, H, b1_sb, ag2h_in, ald2sh)
            nc.gpsimd.collective_compute(
                "AllGather", mybir.AluOpType.bypass, replica_groups=rg,
                ins=[ag2h_in.ap().opt()], outs=[h2tab.ap().opt()])
            if debug_taps:
                nc.sync.dma_start(out=dbg["dbg_h2"][:, :],
                                  in_=h2tab[0:P, 0:D + H])
            # layer 2
            edge_layer(2, h2tab, ("dram", ald2sh), D, H,
                       w3_sb, out_f, heads3, b2_sb, ag3h_in, ald3sh)
            nc.gpsimd.collective_compute(
                "AllGather", mybir.AluOpType.bypass, replica_groups=rg,
                ins=[ag3h_in.ap().opt()], outs=[h3tab.ap().opt()])
            # layer 3
            edge_layer(3, h3tab, ("dram", ald3sh), out_f, heads3,
                       None, 0, 1, b3_sb, None, None)
    nc.finalize()
    return nc


# ----------------------------------------------------------------------------
# host entry point
# ----------------------------------------------------------------------------
def prepare_inputs(x, edge_index, edge_weight, W1, a_src1, a_dst1, b1,
                   W2, a_src2, a_dst2, b2, W3, a_src3, a_dst3, b3,
                   npad, cores):
    """Returns (in_maps, perm, Ks, offs, S)."""
    x = np.asarray(x, np.float32)
    W1 = np.asarray(W1, np.float32)
    W2 = np.asarray(W2, np.float32)
    W3 = np.asarray(W3, np.float32)
    n_nodes, in_f = x.shape
    d1 = W1.shape[1]
    out_f = W3.shape[1]
    heads = np.asarray(a_src1).shape[0]
    hid = d1 // heads

    perm, Ks, offs, gidx, logew = build_schedule(
        edge_index[0], edge_index[1], edge_weight, n_nodes, npad, cores)

    xp = np.zeros((npad, in_f), np.float32)
    xp[perm[:n_nodes]] = x
    xTt = _np_bf16(xp.T.reshape(in_f, npad // P, P).transpose(1, 0, 2))

    def wcat(W, a_s, a_d, h, c):
        wa = (W.reshape(W.shape[0], h, c) * np.asarray(a_s)[None]).sum(-1)
        wd = (W.reshape(W.shape[0], h, c) * np.asarray(a_d)[None]).sum(-1)
        pad = np.zeros((W.shape[0], 4 - wa.shape[1]), np.float32)
        return np.concatenate([W, wa, pad, wd, pad], axis=1)

    w1full = wcat(W1, a_src1, a_dst1, heads, hid)          # [256, 136]
    w1cat = _np_bf16(w1full.reshape(2, P, d1 + 8))
    w2cat = _np_bf16(wcat(W2, a_src2, a_dst2, heads, hid))  # [128, 136]
    w3cat = _np_bf16(wcat(W3, a_src3, a_dst3, 1, out_f))    # [128, 72]

    nc_rows = npad // cores
    wpc = nc_rows // P
    in_maps = []
    for c in range(cores):
        base = c * nc_rows
        ia = (base + np.arange(wpc)[None, :] * P +
              np.arange(P)[:, None]).astype(np.int32)
        in_maps.append(dict(
            xTt=xTt, w1cat=w1cat, w2cat=w2cat, w3cat=w3cat,
            b1row=np.asarray(b1, np.float32).reshape(1, -1),
            b2row=np.asarray(b2, np.float32).reshape(1, -1),
            b3row=np.asarray(b3, np.float32).reshape(1, -1),
            idxv=gidx[c], logew=logew[c], idx_ald=ia,
        ))
    return in_maps, perm, Ks, offs


def kernel(**inputs):
    npad = 50176
    in_maps, perm, Ks, offs = prepare_inputs(
        npad=npad, cores=CORES, **inputs)
    S = int(offs[-1])
    nc = build_program(npad, Ks, offs, S, IN, HEADS * HID, OUT, 1, CORES)

    from concourse.bass_utils import run_bass_kernel_spmd
    res = run_bass_kernel_spmd(nc, in_maps, core_ids=list(range(CORES)))
    shards = [res.results[c]["out"] for c in range(CORES)]
    full = np.concatenate(shards, axis=0)       # [npad, OUT] in new-id order
    return full[perm[:N]].astype(np.float32)


# revision 30
# speedup vs baseline: 1.1437x; 1.1205x over previous
"""3-layer GAT on 8 TRN2 NeuronCores.

Strategy (1D vertex-cut, dst-sharded):
  * Nodes are permuted: degree-sorted, dealt round-robin to 8 cores, so each
    core owns a contiguous range of NC=6272 "new" node ids whose windows of
    128 consecutive ids have near-uniform in-degree.
  * Per (core, window) the edge list is laid out as K[w] "slots" x 128 dst
    rows (shared K schedule across cores, padding slots have log_ew=-1e30).
  * Per layer, per window: indirect-DMA gather of h[src] rows (bf16) and
    als[src] (f32), attention weight w = exp(leaky(als+ald)+log_ew),
    rhs = [V*w | w], K identity-lhsT matmuls accumulate [128, F+H] in PSUM
    (weighted segment-sum + denominator), normalize, bias(+ReLU).
  * Next-layer tables h'=relu(o)@W', als'=o@(W'·a) are produced per window
    (PE transpose + one matmul) and AllGather'd across cores between layers.
  * segment_max is omitted: logits are bounded (|logit| < ~8), exp is safe
    in f32, and softmax is shift-invariant, so this is mathematically
    identical to the reference.
"""
import numpy as np

# problem constants (hardcoded per harness contract)
N, E, IN, HID, HEADS, OUT = 50000, 800000, 256, 32, 4, 64
SLOPE = 0.2
CORES = 8
P = 128


# ----------------------------------------------------------------------------
# host-side schedule construction (index data only)
# ----------------------------------------------------------------------------
def build_schedule(src, dst, ew, n_nodes, npad, cores):
    """Returns node permutation + per-core slot arrays.

    perm: old->new node id (len npad); Ks: [W] slots per window;
    gidx: [cores, 128, S] int32 src new-ids; logew: [cores, 128, S] f32.
    """
    nc_rows = npad // cores
    wpc = nc_rows // P
    src = np.asarray(src, np.int64)
    dst = np.asarray(dst, np.int64)
    ew = np.asarray(ew, np.float32)

    deg = np.bincount(dst, minlength=npad)
    order = np.argsort(-deg, kind="stable")          # ranks -> old id
    perm = np.empty(npad, np.int64)
    ranks = np.arange(npad)
    perm[order] = (ranks % cores) * nc_rows + ranks // cores

    nsrc = perm[src]
    ndst = perm[dst]
    eorder = np.argsort(ndst, kind="stable")
    nsrc_s = nsrc[eorder].astype(np.int32)
    ndst_s = ndst[eorder]
    ew_s = ew[eorder]

    counts = np.bincount(ndst_s, minlength=npad)
    starts = np.zeros(npad + 1, np.int64)
    np.cumsum(counts, out=starts[1:])
    rank_in_dst = np.arange(len(ndst_s)) - starts[ndst_s]

    Ks = counts.reshape(cores, wpc, P).max(axis=(0, 2))
    Ks = np.maximum(Ks, 1).astype(np.int64)
    offs = np.zeros(wpc + 1, np.int64)
    np.cumsum(Ks, out=offs[1:])
    S = int(offs[-1])

    core_e = ndst_s // nc_rows
    loc = ndst_s % nc_rows
    w_e = loc // P
    p_e = loc % P
    col = offs[w_e] + rank_in_dst

    gidx = np.zeros((cores, P, S), np.int32)
    logew = np.full((cores, P, S), -1e30, np.float32)
    flat = (core_e * P + p_e) * S + col
    gidx.reshape(-1)[flat] = nsrc_s
    with np.errstate(divide="ignore"):
        logew.reshape(-1)[flat] = np.log(np.maximum(ew_s, 0.0)).astype(np.float32)
    return perm, Ks, offs, gidx, logew


def _np_bf16(x):
    import ml_dtypes
    return np.asarray(x, np.float32).astype(ml_dtypes.bfloat16)


# ----------------------------------------------------------------------------
# device program
# ----------------------------------------------------------------------------
def build_program(npad, Ks, offs, S, in_f, hid_heads, out_f, heads3, cores,
                  enable_asserts=False, debug_taps=False):
    """Build the SPMD Bacc program. Shapes:
      xTt   [NT, in_f, 128] bf16   (transposed x, node-tile blocks)
      w1cat [in_f//128, 128, hid_heads+8] bf16
      w2cat [hid_heads, hid_heads+8] bf16
      w3cat [hid_heads, out_f+2] bf16
      b1row/b2row [1, hid_heads] f32 ; b3row [1, out_f] f32
      idxv  [128, S] int32 ; logew [128, S] f32 ; idx_ald [128, WPC] int32
      out   [NC, out_f] f32 (per-core shard)
    """
    import concourse.bacc as bacc
    import concourse.bass as bass
    import concourse.mybir as mybir
    from concourse.masks import make_identity
    from concourse.tile import TileContext

    F32, BF16, I32 = mybir.dt.float32, mybir.dt.bfloat16, mybir.dt.int32
    AF = mybir.ActivationFunctionType
    ALU = mybir.AluOpType

    nc_rows = npad // cores
    wpc = nc_rows // P
    nt = npad // P
    ic = in_f // P                       # input chunk count (2)
    D = hid_heads                         # 128
    H = HEADS
    wmax = int(max(Ks))

    nc = bacc.Bacc("TRN2", target_bir_lowering=False, debug=False,
                   enable_asserts=enable_asserts, num_devices=cores)

    xTt = nc.dram_tensor("xTt", [nt, in_f, P], BF16, kind="ExternalInput")
    w1cat = nc.dram_tensor("w1cat", [ic, P, D + 8], BF16, kind="ExternalInput")
    w2cat = nc.dram_tensor("w2cat", [D, D + 8], BF16, kind="ExternalInput")
    w3cat = nc.dram_tensor("w3cat", [D, out_f + 8], BF16, kind="ExternalInput")
    b1row = nc.dram_tensor("b1row", [1, D], F32, kind="ExternalInput")
    b2row = nc.dram_tensor("b2row", [1, D], F32, kind="ExternalInput")
    b3row = nc.dram_tensor("b3row", [1, out_f], F32, kind="ExternalInput")
    idxv = nc.dram_tensor("idxv", [P, S], I32, kind="ExternalInput")
    logew_d = nc.dram_tensor("logew", [P, S], F32, kind="ExternalInput")
    idx_ald = nc.dram_tensor("idx_ald", [P, wpc], I32, kind="ExternalInput")
    out_d = nc.dram_tensor("out", [nc_rows, out_f], F32, kind="ExternalOutput")
    dbg = {}
    if debug_taps:
        K0 = int(Ks[0])
        for nm, shp, dt in [
                ("dbg_h1", [P, D + H], F32),
                ("dbg_ald1", [P, H], F32),
                ("dbg_V", [P, K0, D + H], F32),
                ("dbg_wt", [P, K0, H], F32), ("dbg_acc", [P, D + H], F32),
                ("dbg_o", [P, D], F32), ("dbg_h2", [P, D + H], F32)]:
            dbg[nm] = nc.dram_tensor(nm, shp, dt, kind="ExternalOutput")

    # internal DRAM
    h1tab = nc.dram_tensor("h1tab", [npad, D + H], F32)
    ald1tab = nc.dram_tensor("ald1tab", [npad, H], F32)
    ag2h_in = nc.dram_tensor("ag2h_in", [nc_rows, D + H], F32)
    ald2sh = nc.dram_tensor("ald2sh", [nc_rows, H], F32)
    h2tab = nc.dram_tensor("h2tab", [npad, D + H], F32, addr_space="Shared")
    ag3h_in = nc.dram_tensor("ag3h_in", [nc_rows, out_f + heads3], F32)
    ald3sh = nc.dram_tensor("ald3sh", [nc_rows, heads3], F32)
    h3tab = nc.dram_tensor("h3tab", [npad, out_f + heads3], F32,
                           addr_space="Shared")

    rg = [list(range(cores))]

    with TileContext(nc) as tc:
        with tc.tile_pool(name="const", bufs=1) as cp, \
             tc.tile_pool(name="xin", bufs=3) as xp, \
             tc.tile_pool(name="work", bufs=3) as wk, \
             tc.tile_pool(name="small", bufs=3) as sm, \
             tc.tile_pool(name="ps", bufs=2, space="PSUM") as pp, \
             tc.tile_pool(name="ps2", bufs=2, space="PSUM") as pp2:

            ident_b = cp.tile([P, P], BF16, tag="identb")
            make_identity(nc, ident_b[:])
            ident_f = cp.tile([P, P], F32, tag="identf")
            make_identity(nc, ident_f[:])

            w1_sb = cp.tile([P, ic, D + 8], BF16, tag="w1")
            nc.sync.dma_start(out=w1_sb[:],
                              in_=w1cat.ap().rearrange("c p f -> p c f"))
            w2_sb = cp.tile([P, D + 8], BF16, tag="w2")
            nc.sync.dma_start(out=w2_sb[:], in_=w2cat[:, :])
            w3_sb = cp.tile([P, out_f + 8], BF16, tag="w3")
            nc.sync.dma_start(out=w3_sb[:], in_=w3cat[:, :])
            b1_sb = cp.tile([P, D], F32, tag="b1")
            nc.sync.dma_start(out=b1_sb[:], in_=b1row.ap().to_broadcast((P, D)))
            b2_sb = cp.tile([P, D], F32, tag="b2")
            nc.sync.dma_start(out=b2_sb[:], in_=b2row.ap().to_broadcast((P, D)))
            b3_sb = cp.tile([P, out_f], F32, tag="b3")
            nc.sync.dma_start(out=b3_sb[:],
                              in_=b3row.ap().to_broadcast((P, out_f)))
            idx_sb = cp.tile([P, S], I32, tag="idx")
            nc.sync.dma_start(out=idx_sb[:], in_=idxv[:, :])
            lew_sb = cp.tile([P, S], F32, tag="lew")
            nc.sync.dma_start(out=lew_sb[:], in_=logew_d[:, :])
            idxa_sb = cp.tile([P, wpc], I32, tag="idxa")
            nc.sync.dma_start(out=idxa_sb[:], in_=idx_ald[:, :])

            # ---------------- stage A: layer-1 tables (full, local) --------
            GA = 8
            assert nt % GA == 0
            for t0 in range(0, nt, GA):
                xt = xp.tile([P, GA, ic, P], BF16, tag="xt")
                nc.sync.dma_start(
                    out=xt[:],
                    in_=xTt[t0:t0 + GA].rearrange("g (c p) n -> p g c n",
                                                  p=P))
                h_sb = sm.tile([P, GA, D + H], F32, tag="hA")
                a_sb = sm.tile([P, GA, H], F32, tag="aA")
                for g in range(GA):
                    ps = pp.tile([P, D + 8], F32, tag="psA")
                    for c in range(ic):
                        nc.tensor.matmul(ps[:], lhsT=xt[:, g, c, :],
                                         rhs=w1_sb[:, c, :],
                                         start=(c == 0), stop=(c == ic - 1))
                    nc.scalar.activation(h_sb[:, g, :], ps[:, 0:D + H],
                                         AF.Copy)
                    nc.vector.tensor_copy(out=a_sb[:, g, :],
                                          in_=ps[:, D + 4:D + 4 + H])
                nc.sync.dma_start(
                    out=h1tab[t0 * P:(t0 + GA) * P, :].rearrange(
                        "(g p) f -> p g f", p=P),
                    in_=h_sb[:])
                nc.sync.dma_start(
                    out=ald1tab[t0 * P:(t0 + GA) * P, :].rearrange(
                        "(g p) f -> p g f", p=P),
                    in_=a_sb[:])

            # gather own ald windows into SBUF (core-dependent rows via data)
            ald1_all = cp.tile([P, wpc, H], F32, tag="ald1all")
            for w in range(wpc):
                nc.gpsimd.indirect_dma_start(
                    out=ald1_all[:, w, :], out_offset=None, in_=ald1tab[:, :],
                    in_offset=bass.IndirectOffsetOnAxis(
                        ap=idxa_sb[:, w:w + 1], axis=0))

            # ---------------- edge pass ------------------------------------
            def edge_layer(layer, tab, ald_src, F_in, H_l, wnext_sb,
                           F_next, H_n, bias_sb, agh, aldnext):
                """One GAT layer over all windows.
                ald_src: ("sbuf", tile) or ("dram", tensor)."""
                for w in range(wpc):
                    K = int(Ks[w])
                    off = int(offs[w])
                    r0 = w * P
                    V = wk.tile([P, K, F_in + H_l], F32, tag="V")
                    for k in range(K):
                        nc.gpsimd.indirect_dma_start(
                            out=V[:, k, :], out_offset=None, in_=tab[:, :],
                            in_offset=bass.IndirectOffsetOnAxis(
                                ap=idx_sb[:, off + k:off + k + 1], axis=0))
                    ALS = V[:, :, F_in:F_in + H_l]
                    if ald_src[0] == "sbuf":
                        ald_w = ald_src[1][:, w, :]
                    else:
                        ald_t = sm.tile([P, H_l], F32, tag="aldw")
                        nc.sync.dma_start(out=ald_t[:],
                                          in_=ald_src[1][r0:r0 + P, :])
                        ald_w = ald_t[:]
                    # logit = ALS + ald (bcast over K) + logew (bcast over H)
                    logit = wk.tile([P, K, H_l], F32, tag="logit")
                    ald_b = bass.AP(ald_w.tensor, ald_w.offset,
                                    [ald_w.ap[0], [0, K], [1, H_l]])
                    nc.vector.tensor_add(out=logit[:], in0=ALS, in1=ald_b)
                    lew_ap = lew_sb[:, off:off + K]
                    lew_b = bass.AP(lew_ap.tensor, lew_ap.offset,
                                    [lew_ap.ap[0], [1, K], [0, H_l]])
                    nc.vector.tensor_add(out=logit[:], in0=logit[:],
                                         in1=lew_b)
                    # w = exp(max(0.2*logit, logit))
                    wt = wk.tile([P, K, H_l], F32, tag="wt")
                    nc.vector.scalar_tensor_tensor(
                        out=wt[:], in0=logit[:], scalar=SLOPE, in1=logit[:],
                        op0=ALU.mult, op1=ALU.max)
                    nc.scalar.activation(wt[:], wt[:], AF.Exp)
                    if debug_taps and layer == 1 and w == 0:
                        nc.sync.dma_start(out=dbg["dbg_V"][:, :, :], in_=V[:])
                        nc.sync.dma_start(out=dbg["dbg_wt"][:, :, :],
                                          in_=wt[:])
                    wtb = wk.tile([P, K, H_l], BF16, tag="wtb")
                    nc.vector.tensor_copy(out=wtb[:], in_=wt[:])
                    # rhs = [V*w | w]
                    rhs = wk.tile([P, K, F_in + H_l], BF16, tag="rhs")
                    ch = F_in // H_l
                    wrep = bass.AP(wtb.tensor, wtb[:].offset,
                                   [wtb[:].ap[0], [H_l, K], [1, H_l], [0, ch]])
                    nc.vector.tensor_mul(out=rhs[:, :, 0:F_in],
                                         in0=V[:, :, 0:F_in], in1=wrep)
                    nc.vector.tensor_copy(out=rhs[:, :, F_in:F_in + H_l],
                                          in_=wtb[:])
                    acc = pp.tile([P, F_in + H_l], F32, tag="acc")
                    for k in range(K):
                        nc.tensor.matmul(acc[:], lhsT=ident_b[:],
                                         rhs=rhs[:, k, :],
                                         start=(k == 0), stop=(k == K - 1))
                    if debug_taps and layer == 1 and w == 0:
                        acc_dbg = sm.tile([P, F_in + H_l], F32, tag="accdbg")
                        nc.scalar.activation(acc_dbg[:], acc[:], AF.Copy)
                        nc.sync.dma_start(out=dbg["dbg_acc"][:, :],
                                          in_=acc_dbg[:])
                    den = sm.tile([P, H_l], F32, tag="den")
                    nc.vector.tensor_scalar_add(den[:],
                                                acc[:, F_in:F_in + H_l],
                                                1e-16)
                    rec = sm.tile([P, H_l], F32, tag="rec")
                    nc.vector.reciprocal(rec[:], den[:])
                    o = sm.tile([P, F_in], F32, tag="o")
                    rrep = bass.AP(rec.tensor, rec[:].offset,
                                   [rec[:].ap[0], [1, H_l], [0, ch]])
                    nc.vector.tensor_mul(out=o[:], in0=acc[:, 0:F_in],
                                         in1=rrep)
                    nc.vector.tensor_add(out=o[:], in0=o[:], in1=bias_sb[:])
                    if debug_taps and layer == 1 and w == 0:
                        nc.sync.dma_start(out=dbg["dbg_o"][:, :], in_=o[:])
                    if layer < 3:
                        nc.scalar.activation(o[:], o[:], AF.Relu)
                        # next-layer table rows for this window
                        oT = pp2.tile([P, P], F32, tag="oT")
                        nc.tensor.transpose(out=oT[:], in_=o[:],
                                            identity=ident_f[:])
                        oT_sb = sm.tile([P, P], BF16, tag="oTsb")
                        nc.scalar.activation(oT_sb[:], oT[:], AF.Copy)
                        hn = pp2.tile([P, F_next + 8], F32, tag="hn")
                        nc.tensor.matmul(hn[:], lhsT=oT_sb[:],
                                         rhs=wnext_sb[:, 0:F_next + 8],
                                         start=True, stop=True)
                        hn_sb = sm.tile([P, F_next + H_n], F32, tag="hnsb")
                        nc.scalar.activation(hn_sb[:, 0:F_next],
                                             hn[:, 0:F_next], AF.Copy)
                        nc.vector.tensor_copy(
                            out=hn_sb[:, F_next:F_next + H_n],
                            in_=hn[:, F_next:F_next + H_n])
                        nc.sync.dma_start(out=agh[r0:r0 + P, :], in_=hn_sb[:])
                        an_sb = sm.tile([P, H_n], F32, tag="ansb")
                        nc.vector.tensor_copy(
                            out=an_sb[:],
                            in_=hn[:, F_next + 4:F_next + 4 + H_n])
                        nc.sync.dma_start(out=aldnext[r0:r0 + P, :],
                                          in_=an_sb[:])
                    else:
                        nc.sync.dma_start(out=out_d[r0:r0 + P, :], in_=o[:])

            if debug_taps:
                nc.sync.dma_start(out=dbg["dbg_h1"][:, :],
                                  in_=h1tab[0:P, 0:D + H])
                nc.sync.dma_start(out=dbg["dbg_ald1"][:, :],
                                  in_=ald1tab[0:P, :])
            # layer 1
            edge_layer(1, h1tab, ("sbuf", ald1_all), D, H,
                       w2_sb, D, H, b1_sb, ag2h_in, ald2sh)
            nc.gpsimd.collective_compute(
                "AllGather", mybir.AluOpType.bypass, replica_groups=rg,
                ins=[ag2h_in.ap().opt()], outs=[h2tab.ap().opt()])
            if debug_taps:
                nc.sync.dma_start(out=dbg["dbg_h2"][:, :],
                                  in_=h2tab[0:P, 0:D + H])
            # layer 2
            edge_layer(2, h2tab, ("dram", ald2sh), D, H,
                       w3_sb, out_f, heads3, b2_sb, ag3h_in, ald3sh)
            nc.gpsimd.collective_compute(
                "AllGather", mybir.AluOpType.bypass, replica_groups=rg,
                ins=[ag3h_in.ap().opt()], outs=[h3tab.ap().opt()])
            # layer 3
            edge_layer(3, h3tab, ("dram", ald3sh), out_f, heads3,
                       None, 0, 1, b3_sb, None, None)
    nc.finalize()
    return nc


# ----------------------------------------------------------------------------
# host entry point
# ----------------------------------------------------------------------------
def prepare_inputs(x, edge_index, edge_weight, W1, a_src1, a_dst1, b1,
                   W2, a_src2, a_dst2, b2, W3, a_src3, a_dst3, b3,
                   npad, cores):
    """Returns (in_maps, perm, Ks, offs, S)."""
    x = np.asarray(x, np.float32)
    W1 = np.asarray(W1, np.float32)
    W2 = np.asarray(W2, np.float32)
    W3 = np.asarray(W3, np.float32)
    n_nodes, in_f = x.shape
    d1 = W1.shape[1]
    out_f = W3.shape[1]
    heads = np.asarray(a_src1).shape[0]
    hid = d1 // heads

    perm, Ks, offs, gidx, logew = build_schedule(
        edge_index[0], edge_index[1], edge_weight, n_nodes, npad, cores)

    xp = np.zeros((npad, in_f), np.float32)
    xp[perm[:n_nodes]] = x
    xTt = _np_bf16(xp.T.reshape(in_f, npad // P, P).transpose(1, 0, 2))

    def wcat(W, a_s, a_d, h, c):
        wa = (W.reshape(W.shape[0], h, c) * np.asarray(a_s)[None]).sum(-1)
        wd = (W.reshape(W.shape[0], h, c) * np.asarray(a_d)[None]).sum(-1)
        pad = np.zeros((W.shape[0], 4 - wa.shape[1]), np.float32)
        return np.concatenate([W, wa, pad, wd, pad], axis=1)

    w1full = wcat(W1, a_src1, a_dst1, heads, hid)          # [256, 136]
    w1cat = _np_bf16(w1full.reshape(2, P, d1 + 8))
    w2cat = _np_bf16(wcat(W2, a_src2, a_dst2, heads, hid))  # [128, 136]
    w3cat = _np_bf16(wcat(W3, a_src3, a_dst3, 1, out_f))    # [128, 72]

    nc_rows = npad // cores
    wpc = nc_rows // P
    in_maps = []
    for c in range(cores):
        base = c * nc_rows
        ia = (base + np.arange(wpc)[None, :] * P +
              np.arange(P)[:, None]).astype(np.int32)
        in_maps.append(dict(
            xTt=xTt, w1cat=w1cat, w2cat=w2cat, w3cat=w3cat,
            b1row=np.asarray(b1, np.float32).reshape(1, -1),
            b2row=np.asarray(b2, np.float32).reshape(1, -1),
            b3row=np.asarray(b3, np.float32).reshape(1, -1),
            idxv=gidx[c], logew=logew[c], idx_ald=ia,
        ))
    return in_maps, perm, Ks, offs


def kernel(**inputs):
    npad = 50176
    in_maps, perm, Ks, offs = prepare_inputs(
        npad=npad, cores=CORES, **inputs)
    S = int(offs[-1])
    nc = build_program(npad, Ks, offs, S, IN, HEADS * HID, OUT, 1, CORES)

    from concourse.bass_utils import run_bass_kernel_spmd
    res = run_bass_kernel_spmd(nc, in_maps, core_ids=list(range(CORES)))
    shards = [res.results[c]["out"] for c in range(CORES)]
    full = np.concatenate(shards, axis=0)       # [npad, OUT] in new-id order
    return full[perm[:N]].astype(np.float32)


# revision 33
# speedup vs baseline: 1.1641x; 1.0178x over previous
"""3-layer GAT on 8 TRN2 NeuronCores.

Strategy (1D vertex-cut, dst-sharded):
  * Nodes are permuted: degree-sorted, dealt round-robin to 8 cores, so each
    core owns a contiguous range of NC=6272 "new" node ids whose windows of
    128 consecutive ids have near-uniform in-degree.
  * Per (core, window) the edge list is laid out as K[w] "slots" x 128 dst
    rows (shared K schedule across cores, padding slots have log_ew=-1e30).
  * Per layer, per window: indirect-DMA gather of h[src] rows (bf16) and
    als[src] (f32), attention weight w = exp(leaky(als+ald)+log_ew),
    rhs = [V*w | w], K identity-lhsT matmuls accumulate [128, F+H] in PSUM
    (weighted segment-sum + denominator), normalize, bias(+ReLU).
  * Next-layer tables h'=relu(o)@W', als'=o@(W'·a) are produced per window
    (PE transpose + one matmul) and AllGather'd across cores between layers.
  * segment_max is omitted: logits are bounded (|logit| < ~8), exp is safe
    in f32, and softmax is shift-invariant, so this is mathematically
    identical to the reference.
"""
import numpy as np

# problem constants (hardcoded per harness contract)
N, E, IN, HID, HEADS, OUT = 50000, 800000, 256, 32, 4, 64
SLOPE = 0.2
CORES = 8
P = 128


# ----------------------------------------------------------------------------
# host-side schedule construction (index data only)
# ----------------------------------------------------------------------------
def build_schedule(src, dst, ew, n_nodes, npad, cores):
    """Returns node permutation + per-core slot arrays.

    perm: old->new node id (len npad); Ks: [W] slots per window;
    gidx: [cores, 128, S] int32 src new-ids; logew: [cores, 128, S] f32.
    """
    nc_rows = npad // cores
    wpc = nc_rows // P
    src = np.asarray(src, np.int64)
    dst = np.asarray(dst, np.int64)
    ew = np.asarray(ew, np.float32)

    deg = np.bincount(dst, minlength=npad)
    order = np.argsort(-deg, kind="stable")          # ranks -> old id
    perm = np.empty(npad, np.int64)
    ranks = np.arange(npad)
    perm[order] = (ranks % cores) * nc_rows + ranks // cores

    nsrc = perm[src]
    ndst = perm[dst]
    eorder = np.argsort(ndst, kind="stable")
    nsrc_s = nsrc[eorder].astype(np.int32)
    ndst_s = ndst[eorder]
    ew_s = ew[eorder]

    counts = np.bincount(ndst_s, minlength=npad)
    starts = np.zeros(npad + 1, np.int64)
    np.cumsum(counts, out=starts[1:])
    rank_in_dst = np.arange(len(ndst_s)) - starts[ndst_s]

    Ks = counts.reshape(cores, wpc, P).max(axis=(0, 2))
    Ks = np.maximum(Ks, 1).astype(np.int64)
    offs = np.zeros(wpc + 1, np.int64)
    np.cumsum(Ks, out=offs[1:])
    S = int(offs[-1])

    core_e = ndst_s // nc_rows
    loc = ndst_s % nc_rows
    w_e = loc // P
    p_e = loc % P
    col = offs[w_e] + rank_in_dst

    gidx = np.zeros((cores, P, S), np.int32)
    logew = np.full((cores, P, S), -1e30, np.float32)
    flat = (core_e * P + p_e) * S + col
    gidx.reshape(-1)[flat] = nsrc_s
    with np.errstate(divide="ignore"):
        logew.reshape(-1)[flat] = np.log(np.maximum(ew_s, 0.0)).astype(np.float32)
    return perm, Ks, offs, gidx, logew


def _np_bf16(x):
    import ml_dtypes
    return np.asarray(x, np.float32).astype(ml_dtypes.bfloat16)


# ----------------------------------------------------------------------------
# device program
# ----------------------------------------------------------------------------
def build_program(npad, Ks, offs, S, in_f, hid_heads, out_f, heads3, cores,
                  enable_asserts=False, debug_taps=False):
    """Build the SPMD Bacc program. Shapes:
      xTt   [NT, in_f, 128] bf16   (transposed x, node-tile blocks)
      w1cat [in_f//128, 128, hid_heads+8] bf16
      w2cat [hid_heads, hid_heads+8] bf16
      w3cat [hid_heads, out_f+2] bf16
      b1row/b2row [1, hid_heads] f32 ; b3row [1, out_f] f32
      idxv  [128, S] int32 ; logew [128, S] f32 ; idx_ald [128, WPC] int32
      out   [NC, out_f] f32 (per-core shard)
    """
    import concourse.bacc as bacc
    import concourse.bass as bass
    import concourse.mybir as mybir
    from concourse.masks import make_identity
    from concourse.tile import TileContext

    F32, BF16, I32 = mybir.dt.float32, mybir.dt.bfloat16, mybir.dt.int32
    AF = mybir.ActivationFunctionType
    ALU = mybir.AluOpType

    nc_rows = npad // cores
    wpc = nc_rows // P
    nt = npad // P
    ic = in_f // P                       # input chunk count (2)
    D = hid_heads                         # 128
    H = HEADS
    wmax = int(max(Ks))

    nc = bacc.Bacc("TRN2", target_bir_lowering=False, debug=False,
                   enable_asserts=enable_asserts, num_devices=cores)

    xTt = nc.dram_tensor("xTt", [nt, in_f, P], BF16, kind="ExternalInput")
    w1cat = nc.dram_tensor("w1cat", [ic, P, D + 8], BF16, kind="ExternalInput")
    w2cat = nc.dram_tensor("w2cat", [D, D + 8], BF16, kind="ExternalInput")
    w3cat = nc.dram_tensor("w3cat", [D, out_f + 8], BF16, kind="ExternalInput")
    b1row = nc.dram_tensor("b1row", [1, D], F32, kind="ExternalInput")
    b2row = nc.dram_tensor("b2row", [1, D], F32, kind="ExternalInput")
    b3row = nc.dram_tensor("b3row", [1, out_f], F32, kind="ExternalInput")
    idxv = nc.dram_tensor("idxv", [P, S], I32, kind="ExternalInput")
    logew_d = nc.dram_tensor("logew", [P, S], F32, kind="ExternalInput")
    idx_ald = nc.dram_tensor("idx_ald", [P, wpc], I32, kind="ExternalInput")
    out_d = nc.dram_tensor("out", [nc_rows, out_f], F32, kind="ExternalOutput")
    dbg = {}
    if debug_taps:
        K0 = int(Ks[0])
        for nm, shp, dt in [
                ("dbg_h1", [P, D + H], F32),
                ("dbg_ald1", [P, H], F32),
                ("dbg_V", [P, K0, D + H], F32),
                ("dbg_wt", [P, K0, H], F32), ("dbg_acc", [P, D + H], F32),
                ("dbg_o", [P, D], F32), ("dbg_h2", [P, D + H], F32)]:
            dbg[nm] = nc.dram_tensor(nm, shp, dt, kind="ExternalOutput")

    # internal DRAM
    h1tab = nc.dram_tensor("h1tab", [npad, D + H], F32)
    ald1tab = nc.dram_tensor("ald1tab", [npad, H], F32)
    ag2h_in = nc.dram_tensor("ag2h_in", [nc_rows, D + H], F32)
    ald2sh = nc.dram_tensor("ald2sh", [nc_rows, H], F32)
    h2tab = nc.dram_tensor("h2tab", [npad, D + H], F32, addr_space="Shared")
    ag3h_in = nc.dram_tensor("ag3h_in", [nc_rows, out_f + heads3], F32)
    ald3sh = nc.dram_tensor("ald3sh", [nc_rows, heads3], F32)
    h3tab = nc.dram_tensor("h3tab", [npad, out_f + heads3], F32,
                           addr_space="Shared")

    rg = [list(range(cores))]

    with TileContext(nc) as tc:
        with tc.tile_pool(name="const", bufs=1) as cp, \
             tc.tile_pool(name="xin", bufs=3) as xp, \
             tc.tile_pool(name="work", bufs=3) as wk, \
             tc.tile_pool(name="small", bufs=3) as sm, \
             tc.tile_pool(name="ps", bufs=2, space="PSUM") as pp, \
             tc.tile_pool(name="ps2", bufs=2, space="PSUM") as pp2:

            ident_b = cp.tile([P, P], BF16, tag="identb")
            make_identity(nc, ident_b[:])
            ident_f = cp.tile([P, P], F32, tag="identf")
            make_identity(nc, ident_f[:])

            w1_sb = cp.tile([P, ic, D + 8], BF16, tag="w1")
            nc.sync.dma_start(out=w1_sb[:],
                              in_=w1cat.ap().rearrange("c p f -> p c f"))
            w2_sb = cp.tile([P, D + 8], BF16, tag="w2")
            nc.sync.dma_start(out=w2_sb[:], in_=w2cat[:, :])
            w3_sb = cp.tile([P, out_f + 8], BF16, tag="w3")
            nc.sync.dma_start(out=w3_sb[:], in_=w3cat[:, :])
            b1_sb = cp.tile([P, D], F32, tag="b1")
            nc.sync.dma_start(out=b1_sb[:], in_=b1row.ap().to_broadcast((P, D)))
            b2_sb = cp.tile([P, D], F32, tag="b2")
            nc.sync.dma_start(out=b2_sb[:], in_=b2row.ap().to_broadcast((P, D)))
            b3_sb = cp.tile([P, out_f], F32, tag="b3")
            nc.sync.dma_start(out=b3_sb[:],
                              in_=b3row.ap().to_broadcast((P, out_f)))
            idx_sb = cp.tile([P, S], I32, tag="idx")
            nc.sync.dma_start(out=idx_sb[:], in_=idxv[:, :])
            lew_sb = cp.tile([P, S], F32, tag="lew")
            nc.sync.dma_start(out=lew_sb[:], in_=logew_d[:, :])
            idxa_sb = cp.tile([P, wpc], I32, tag="idxa")
            nc.sync.dma_start(out=idxa_sb[:], in_=idx_ald[:, :])

            # ---------------- stage A: layer-1 tables (full, local) --------
            GA = 14
            assert nt % GA == 0
            for t0 in range(0, nt, GA):
                xt = xp.tile([P, GA, ic, P], BF16, tag="xt")
                nc.sync.dma_start(
                    out=xt[:],
                    in_=xTt[t0:t0 + GA].rearrange("g (c p) n -> p g c n",
                                                  p=P))
                h_sb = sm.tile([P, GA, D + H], F32, tag="hA")
                a_sb = sm.tile([P, GA, H], F32, tag="aA")
                for g in range(GA):
                    ps = pp.tile([P, D + 8], F32, tag="psA")
                    for c in range(ic):
                        nc.tensor.matmul(ps[:], lhsT=xt[:, g, c, :],
                                         rhs=w1_sb[:, c, :],
                                         start=(c == 0), stop=(c == ic - 1))
                    nc.scalar.activation(h_sb[:, g, :], ps[:, 0:D + H],
                                         AF.Copy)
                    nc.vector.tensor_copy(out=a_sb[:, g, :],
                                          in_=ps[:, D + 4:D + 4 + H])
                nc.sync.dma_start(
                    out=h1tab[t0 * P:(t0 + GA) * P, :].rearrange(
                        "(g p) f -> p g f", p=P),
                    in_=h_sb[:])
                nc.sync.dma_start(
                    out=ald1tab[t0 * P:(t0 + GA) * P, :].rearrange(
                        "(g p) f -> p g f", p=P),
                    in_=a_sb[:])

            # gather own ald windows into SBUF (core-dependent rows via data)
            ald1_all = cp.tile([P, wpc, H], F32, tag="ald1all")
            for w in range(wpc):
                nc.gpsimd.indirect_dma_start(
                    out=ald1_all[:, w, :], out_offset=None, in_=ald1tab[:, :],
                    in_offset=bass.IndirectOffsetOnAxis(
                        ap=idxa_sb[:, w:w + 1], axis=0))

            # ---------------- edge pass ------------------------------------
            def edge_layer(layer, tab, ald_src, F_in, H_l, wnext_sb,
                           F_next, H_n, bias_sb, agh, aldnext):
                """One GAT layer over all windows.
                ald_src: ("sbuf", tile) or ("dram", tensor)."""
                for w in range(wpc):
                    K = int(Ks[w])
                    off = int(offs[w])
                    r0 = w * P
                    V = wk.tile([P, K, F_in + H_l], F32, tag="V")
                    for k in range(K):
                        nc.gpsimd.indirect_dma_start(
                            out=V[:, k, :], out_offset=None, in_=tab[:, :],
                            in_offset=bass.IndirectOffsetOnAxis(
                                ap=idx_sb[:, off + k:off + k + 1], axis=0))
                    ALS = V[:, :, F_in:F_in + H_l]
                    if ald_src[0] == "sbuf":
                        ald_w = ald_src[1][:, w, :]
                    else:
                        ald_t = sm.tile([P, H_l], F32, tag="aldw")
                        nc.sync.dma_start(out=ald_t[:],
                                          in_=ald_src[1][r0:r0 + P, :])
                        ald_w = ald_t[:]
                    # logit = ALS + ald (bcast over K) + logew (bcast over H)
                    logit = wk.tile([P, K, H_l], F32, tag="logit")
                    ald_b = bass.AP(ald_w.tensor, ald_w.offset,
                                    [ald_w.ap[0], [0, K], [1, H_l]])
                    nc.vector.tensor_add(out=logit[:], in0=ALS, in1=ald_b)
                    lew_ap = lew_sb[:, off:off + K]
                    lew_b = bass.AP(lew_ap.tensor, lew_ap.offset,
                                    [lew_ap.ap[0], [1, K], [0, H_l]])
                    nc.vector.tensor_add(out=logit[:], in0=logit[:],
                                         in1=lew_b)
                    # w = exp(max(0.2*logit, logit))
                    wt = wk.tile([P, K, H_l], F32, tag="wt")
                    nc.vector.scalar_tensor_tensor(
                        out=wt[:], in0=logit[:], scalar=SLOPE, in1=logit[:],
                        op0=ALU.mult, op1=ALU.max)
                    nc.scalar.activation(wt[:], wt[:], AF.Exp)
                    if debug_taps and layer == 1 and w == 0:
                        nc.sync.dma_start(out=dbg["dbg_V"][:, :, :], in_=V[:])
                        nc.sync.dma_start(out=dbg["dbg_wt"][:, :, :],
                                          in_=wt[:])
                    wtb = wk.tile([P, K, H_l], BF16, tag="wtb")
                    nc.vector.tensor_copy(out=wtb[:], in_=wt[:])
                    # rhs = [V*w | w]
                    rhs = wk.tile([P, K, F_in + H_l], BF16, tag="rhs")
                    ch = F_in // H_l
                    wrep = bass.AP(wtb.tensor, wtb[:].offset,
                                   [wtb[:].ap[0], [H_l, K], [1, H_l], [0, ch]])
                    nc.vector.tensor_mul(out=rhs[:, :, 0:F_in],
                                         in0=V[:, :, 0:F_in], in1=wrep)
                    nc.vector.tensor_copy(out=rhs[:, :, F_in:F_in + H_l],
                                          in_=wtb[:])
                    acc = pp.tile([P, F_in + H_l], F32, tag="acc")
                    for k in range(K):
                        nc.tensor.matmul(acc[:], lhsT=ident_b[:],
                                         rhs=rhs[:, k, :],
                                         start=(k == 0), stop=(k == K - 1))
                    if debug_taps and layer == 1 and w == 0:
                        acc_dbg = sm.tile([P, F_in + H_l], F32, tag="accdbg")
                        nc.scalar.activation(acc_dbg[:], acc[:], AF.Copy)
                        nc.sync.dma_start(out=dbg["dbg_acc"][:, :],
                                          in_=acc_dbg[:])
                    den = sm.tile([P, H_l], F32, tag="den")
                    nc.vector.tensor_scalar_add(den[:],
                                                acc[:, F_in:F_in + H_l],
                                                1e-16)
                    rec = sm.tile([P, H_l], F32, tag="rec")
                    nc.vector.reciprocal(rec[:], den[:])
                    o = sm.tile([P, F_in], F32, tag="o")
                    rrep = bass.AP(rec.tensor, rec[:].offset,
                                   [rec[:].ap[0], [1, H_l], [0, ch]])
                    nc.vector.tensor_mul(out=o[:], in0=acc[:, 0:F_in],
                                         in1=rrep)
                    nc.vector.tensor_add(out=o[:], in0=o[:], in1=bias_sb[:])
                    if debug_taps and layer == 1 and w == 0:
                        nc.sync.dma_start(out=dbg["dbg_o"][:, :], in_=o[:])
                    if layer < 3:
                        nc.scalar.activation(o[:], o[:], AF.Relu)
                        # next-layer table rows for this window
                        oT = pp2.tile([P, P], F32, tag="oT")
                        nc.tensor.transpose(out=oT[:], in_=o[:],
                                            identity=ident_f[:])
                        oT_sb = sm.tile([P, P], BF16, tag="oTsb")
                        nc.scalar.activation(oT_sb[:], oT[:], AF.Copy)
                        hn = pp2.tile([P, F_next + 8], F32, tag="hn")
                        nc.tensor.matmul(hn[:], lhsT=oT_sb[:],
                                         rhs=wnext_sb[:, 0:F_next + 8],
                                         start=True, stop=True)
                        hn_sb = sm.tile([P, F_next + H_n], F32, tag="hnsb")
                        nc.scalar.activation(hn_sb[:, 0:F_next],
                                             hn[:, 0:F_next], AF.Copy)
                        nc.vector.tensor_copy(
                            out=hn_sb[:, F_next:F_next + H_n],
                            in_=hn[:, F_next:F_next + H_n])
                        nc.sync.dma_start(out=agh[r0:r0 + P, :], in_=hn_sb[:])
                        an_sb = sm.tile([P, H_n], F32, tag="ansb")
                        nc.vector.tensor_copy(
                            out=an_sb[:],
                            in_=hn[:, F_next + 4:F_next + 4 + H_n])
                        nc.sync.dma_start(out=aldnext[r0:r0 + P, :],
                                          in_=an_sb[:])
                    else:
                        nc.sync.dma_start(out=out_d[r0:r0 + P, :], in_=o[:])

            if debug_taps:
                nc.sync.dma_start(out=dbg["dbg_h1"][:, :],
                                  in_=h1tab[0:P, 0:D + H])
                nc.sync.dma_start(out=dbg["dbg_ald1"][:, :],
                                  in_=ald1tab[0:P, :])
            # layer 1
            edge_layer(1, h1tab, ("sbuf", ald1_all), D, H,
                       w2_sb, D, H, b1_sb, ag2h_in, ald2sh)
            nc.gpsimd.collective_compute(
                "AllGather", mybir.AluOpType.bypass, replica_groups=rg,
                ins=[ag2h_in.ap().opt()], outs=[h2tab.ap().opt()])
            if debug_taps:
                nc.sync.dma_start(out=dbg["dbg_h2"][:, :],
                                  in_=h2tab[0:P, 0:D + H])
            # layer 2
            edge_layer(2, h2tab, ("dram", ald2sh), D, H,
                       w3_sb, out_f, heads3, b2_sb, ag3h_in, ald3sh)
            nc.gpsimd.collective_compute(
                "AllGather", mybir.AluOpType.bypass, replica_groups=rg,
                ins=[ag3h_in.ap().opt()], outs=[h3tab.ap().opt()])
            # layer 3
            edge_layer(3, h3tab, ("dram", ald3sh), out_f, heads3,
                       None, 0, 1, b3_sb, None, None)
    nc.finalize()
    return nc


# ----------------------------------------------------------------------------
# host entry point
# ----------------------------------------------------------------------------
def prepare_inputs(x, edge_index, edge_weight, W1, a_src1, a_dst1, b1,
                   W2, a_src2, a_dst2, b2, W3, a_src3, a_dst3, b3,
                   npad, cores):
    """Returns (in_maps, perm, Ks, offs, S)."""
    x = np.asarray(x, np.float32)
    W1 = np.asarray(W1, np.float32)
    W2 = np.asarray(W2, np.float32)
    W3 = np.asarray(W3, np.float32)
    n_nodes, in_f = x.shape
    d1 = W1.shape[1]
    out_f = W3.shape[1]
    heads = np.asarray(a_src1).shape[0]
    hid = d1 // heads

    perm, Ks, offs, gidx, logew = build_schedule(
        edge_index[0], edge_index[1], edge_weight, n_nodes, npad, cores)

    xp = np.zeros((npad, in_f), np.float32)
    xp[perm[:n_nodes]] = x
    xTt = _np_bf16(xp.T.reshape(in_f, npad // P, P).transpose(1, 0, 2))

    def wcat(W, a_s, a_d, h, c):
        wa = (W.reshape(W.shape[0], h, c) * np.asarray(a_s)[None]).sum(-1)
        wd = (W.reshape(W.shape[0], h, c) * np.asarray(a_d)[None]).sum(-1)
        pad = np.zeros((W.shape[0], 4 - wa.shape[1]), np.float32)
        return np.concatenate([W, wa, pad, wd, pad], axis=1)

    w1full = wcat(W1, a_src1, a_dst1, heads, hid)          # [256, 136]
    w1cat = _np_bf16(w1full.reshape(2, P, d1 + 8))
    w2cat = _np_bf16(wcat(W2, a_src2, a_dst2, heads, hid))  # [128, 136]
    w3cat = _np_bf16(wcat(W3, a_src3, a_dst3, 1, out_f))    # [128, 72]

    nc_rows = npad // cores
    wpc = nc_rows // P
    in_maps = []
    for c in range(cores):
        base = c * nc_rows
        ia = (base + np.arange(wpc)[None, :] * P +
              np.arange(P)[:, None]).astype(np.int32)
        in_maps.append(dict(
            xTt=xTt, w1cat=w1cat, w2cat=w2cat, w3cat=w3cat,
            b1row=np.asarray(b1, np.float32).reshape(1, -1),
            b2row=np.asarray(b2, np.float32).reshape(1, -1),
            b3row=np.asarray(b3, np.float32).reshape(1, -1),
            idxv=gidx[c], logew=logew[c], idx_ald=ia,
        ))
    return in_maps, perm, Ks, offs


def kernel(**inputs):
    npad = 50176
    in_maps, perm, Ks, offs = prepare_inputs(
        npad=npad, cores=CORES, **inputs)
    S = int(offs[-1])
    nc = build_program(npad, Ks, offs, S, IN, HEADS * HID, OUT, 1, CORES)

    from concourse.bass_utils import run_bass_kernel_spmd
    res = run_bass_kernel_spmd(nc, in_maps, core_ids=list(range(CORES)))
    shards = [res.results[c]["out"] for c in range(CORES)]
    full = np.concatenate(shards, axis=0)       # [npad, OUT] in new-id order
    return full[perm[:N]].astype(np.float32)
